# revision 1
# baseline (speedup 1.0000x reference)
"""Trainium2 Bass kernel for nn_Net_39230231281866 (dense_cnn).

Network: conv3x3(1->6) -> Taylor-sigmoid -> conv3x3(6->7) -> flatten
         -> fc(4032->128) -> sigmoid -> fc(128->10) -> log_softmax,
batch 8192, data-parallel over 8 NeuronCores (1024 samples/core).

Mapping:
  * conv2+fc1 folded on the host into one dense GEMM W_comb [128, 4056].
  * conv1 = banded-weight matmul (K = input-pixel window, M = 128 padded
    output positions x channels, batch on the moving free dim). Input is
    host pre-windowed (pixel-major, per-slice contiguous) fp16 blocks.
  * Matmuls run in fp16 (fp32 is 4 cycles/row on the PE; fp16 is 1).
    PSUM accumulation stays fp32.
  * Taylor-sigmoid: custom DVE op computes den16(u) = u^4+2u^3+3u^2+3u+3
    where u = (-conv1(x)-b1)/2 (scale folded into the conv weights), then
    a Reciprocal (mostly on ScalarE via the PWP table, a few on VectorE
    via RECIPROCAL_APPROX_FAST to balance engines). s = (24/16)/den16 with
    the 1.5 folded into W_comb.
  * conv1 outputs are written in PSUM-bank pairs so elementwise ops run
    at free-dim 1024, amortizing instruction overheads.
"""

import os
import numpy as np
import ml_dtypes

_B = 8192
_NCORES = 8
_PC = _B // _NCORES
_SLICE = 512
_NSL = _PC // _SLICE

# conv1 output tiling: 26 = 3*8+2 rows, 26 = 7*3+5 cols
_OY_T = [(0, 3), (3, 3), (6, 3), (9, 3), (12, 3), (15, 3), (18, 3), (21, 3), (24, 2)]
_OX_T = [(0, 7), (7, 7), (14, 7), (21, 5)]

# how many of the 36 per-core reciprocal super-ops run on the DVE
# (the rest run on ScalarE) — engine balance knob
_N_DVE_RECIP = 4

LAST_RESULTS = None


def _tiles():
    ts = []
    for (oy0, noy) in _OY_T:
        for (ox0, nox) in _OX_T:
            ts.append(dict(oy0=oy0, noy=noy, ox0=ox0, nox=nox,
                           ky=noy + 2, kx=nox + 2,
                           K=(noy + 2) * (nox + 2), M=noy * nox * 6,
                           cls=(noy, nox)))
    # group by class so psum-pair mates share a bias vector (fewer DVE ops)
    order = {(3, 7): 0, (3, 5): 1, (2, 7): 2, (2, 5): 3}
    ts.sort(key=lambda t: order[t["cls"]])
    return ts


def _host_prep(x, w1, b1, w2, b2, fw1, fb1, fw2, fb2):
    x = np.asarray(x, np.float32)
    w1 = np.asarray(w1, np.float32); b1 = np.asarray(b1, np.float32)
    w2 = np.asarray(w2, np.float32); b2 = np.asarray(b2, np.float32)
    fw1 = np.asarray(fw1, np.float32); fb1 = np.asarray(fb1, np.float32)
    fw2 = np.asarray(fw2, np.float32); fb2 = np.asarray(fb2, np.float32)

    tiles = _tiles()

    # banded conv1 weights, scaled by -1/2 (u = (-conv-b1)/2), M padded to 128
    cls_list = [(3, 7), (3, 5), (2, 7), (2, 5)]
    cls_idx = {c: i for i, c in enumerate(cls_list)}
    w1pack = np.zeros((45, 128 * 4), np.float32)
    biaspack = np.zeros((128, 4), np.float32)
    for cls in cls_list:
        noy, nox = cls
        kx = nox + 2
        ci = cls_idx[cls]
        for oy in range(noy):
            for ox in range(nox):
                for oc in range(6):
                    m = (oy * nox + ox) * 6 + oc
                    biaspack[m, ci] = -0.5 * b1[oc]
                    for dy in range(3):
                        for dx in range(3):
                            k = (oy + dy) * kx + (ox + dx)
                            w1pack[k, 128 * ci + m] = -0.5 * w1[oc, 0, dy, dx]

    # fold conv2 + fc1 -> W_comb [128, 6*26*26] (x1.5: s = 1.5/den16), b_comb
    fw1r = fw1.reshape(128, 7, 24, 24)
    Wc = np.zeros((128, 6, 26, 26), np.float32)
    for dy in range(3):
        for dx in range(3):
            Wc[:, :, dy:dy + 24, dx:dx + 24] += np.einsum(
                "joyx,oi->jiyx", fw1r, w2[:, :, dy, dx], optimize=True)
    b_comb = fb1 + np.einsum("joyx,o->j", fw1r, b2)
    Wc_flat = (1.5 * Wc.reshape(128, 6 * 26 * 26)).astype(np.float32)

    # W_comb columns in conv1-chunk partition order, packed [128, 36*128]
    wcpack = np.zeros((128, 128 * len(tiles)), np.float32)
    for t_i, t in enumerate(tiles):
        rows = []
        for oy in range(t["noy"]):
            for ox in range(t["nox"]):
                for oc in range(6):
                    rows.append((oc * 26 + t["oy0"] + oy) * 26 + t["ox0"] + ox)
        wcpack[:t["M"], 128 * t_i:128 * t_i + 128] = Wc_flat[:, rows].T

    f16 = np.float16
    consts = dict(
        wcpack=wcpack.astype(f16), w1pack=w1pack.astype(f16),
        biaspack=biaspack, cls_idx=cls_idx,
        bcomb=b_comb.reshape(128, 1).astype(np.float32),
        fw2t=np.ascontiguousarray(fw2.T).astype(f16),                   # [128, 10]
        fb2r=np.tile(fb2.reshape(1, 10), (128, 4)).astype(np.float32),  # [128, 40]
    )
    # pre-windowed input: for each conv tile a contiguous [K_t, B] block of
    # pixel-major rows, so each per-slice window DMA is one dense 2D transfer
    x_pm = x.reshape(_B, 784).T.astype(f16)                             # [784, B]
    wins = []
    for t in tiles:
        rows = (np.arange(t["ky"])[:, None] + t["oy0"]) * 28 + \
               (np.arange(t["kx"])[None, :] + t["ox0"])
        wins.append(x_pm[rows.reshape(-1), :])                          # [K_t, B]
    consts["win_offs"] = np.cumsum([0] + [2 * t["K"] for t in tiles])
    return wins, consts, tiles


def _register_taylor_den16():
    import concourse.dve_ops as dve_ops
    if "TAYLOR_DEN16_ANT" in dve_ops._SUB_OPCODE_FOR_NAME:
        return next(o for o in dve_ops.OPS if o.name == "TAYLOR_DEN16_ANT")
    from concourse.dve_spec import Spec, Src0, C0, C1, C2

    # u = in0 + s0;  out = u^4 + 2u^3 + 3u^2 + 3u + 3  ==  (q(t)+48)/16
    u = Src0 + C0
    body = ((((u + C1) * u + C2) * u + C2) * u + C2)

    def _ref(in0, in1, s0, s1, imm2):
        xx = in0.astype(np.float32) + s0
        return (((xx + s1) * xx + imm2) * xx + imm2) * xx + imm2

    op = dve_ops.DveOp(
        "TAYLOR_DEN16_ANT",
        Spec(body=body, reference=_ref),
        subdim=False,
        uops_sha={"v3": "0d84493259836d20", "v4": "be052b2c26b42830"},
    )
    dve_ops.OPS.append(op)
    dve_ops.CUSTOM_DVE_SPECS[op.name] = op.spec
    row = max(dve_ops._SUB_OPCODE_FOR_NAME.values()) + 1
    assert row < 0x20
    dve_ops._SUB_OPCODE_FOR_NAME[op.name] = row
    return op


def _pin_exp_ln_table():
    """Make Exp and Ln resolve only to natural_log_exp_and_others so the
    log_softmax tail costs one table load instead of alternating sets."""
    import concourse.bacc as bacc
    import concourse.mybir as mybir
    if getattr(bacc, "_ant_expln_pinned", False):
        return
    orig = bacc.get_activation_tables
    AF = mybir.ActivationFunctionType

    def patched(arch):
        tabs = {k: set(v) for k, v in orig(arch).items()}
        for name, fns in tabs.items():
            if name != "natural_log_exp_and_others":
                fns.discard(AF.Exp)
                fns.discard(AF.Ln)
        return tabs

    bacc.get_activation_tables = patched
    bacc._ant_expln_pinned = True


def _act_raw(nc, out, in_, func, bias=0.0, scale=1.0):
    """Emit InstActivation directly (used for Reciprocal, which the
    nc.scalar.activation wrapper refuses; measured ~1.2e-5 rel err)."""
    import concourse.mybir as mybir
    eng = nc.scalar
    inputs = [eng.lower_ap(in_)]
    for arg in (bias, scale, 0.0):
        inputs.append(mybir.ImmediateValue(dtype=mybir.dt.float32,
                                           value=float(arg)))
    return eng.add_instruction(mybir.InstActivation(
        name=nc.get_next_instruction_name(), func=func, ins=inputs,
        outs=[eng.lower_ap(out)]))


def _build_program(tiles, cls_idx, win_offs):
    import concourse.bacc as bacc
    import concourse.mybir as mybir
    from concourse.tile import TileContext
    from concourse.tile_rust import add_dep_helper
    from concourse.alu_op_type import AluOpType
    from concourse.dve_ops import RECIP_APPROX_FAST_CONSTS as RC
    import concourse.dve_ops as dve_ops

    f32 = mybir.dt.float32
    f16 = mybir.dt.float16
    AF = mybir.ActivationFunctionType
    taylor_den = _register_taylor_den16()
    recip_fast = next(o for o in dve_ops.OPS if o.name == "RECIPROCAL_APPROX_FAST")
    _pin_exp_ln_table()

    nc = bacc.Bacc()
    n_tiles = len(tiles)
    n_win_rows = int(win_offs[-1]) // 2
    xwin = nc.declare_dram_parameter("xwin", [2 * n_win_rows, _SLICE], f16,
                                     isOutput=False)
    wcpack_d = nc.declare_dram_parameter("wcpack", [128, 128 * n_tiles], f16,
                                         isOutput=False)
    w1pack_d = nc.declare_dram_parameter("w1pack", [45, 512], f16, isOutput=False)
    biaspack_d = nc.declare_dram_parameter("biaspack", [128, 4], f32, isOutput=False)
    bcomb_d = nc.declare_dram_parameter("bcomb", [128, 1], f32, isOutput=False)
    fw2t_d = nc.declare_dram_parameter("fw2t", [128, 10], f16, isOutput=False)
    fb2r_d = nc.declare_dram_parameter("fb2r", [128, 40], f32, isOutput=False)
    out_d = nc.declare_dram_parameter("out", [_PC, 10], f32, isOutput=True)

    with TileContext(nc) as tc:
        with (
            tc.tile_pool(name="const", bufs=1) as cpool,
            tc.tile_pool(name="xw", bufs=8) as xpool,
            tc.tile_pool(name="work", bufs=3) as wpool,
            tc.tile_pool(name="cps", bufs=2, space="PSUM") as cps,
            tc.tile_pool(name="zps", bufs=2, space="PSUM") as zps,
            tc.tile_pool(name="fps", bufs=1, space="PSUM") as fps,
        ):
            w1pack_sb = cpool.tile_from(w1pack_d[:], name="w1pack_sb")
            wcpack_sb = cpool.tile_from(wcpack_d[:], name="wcpack_sb")
            biaspack_sb = cpool.tile_from(biaspack_d[:], name="biaspack_sb")
            bcomb_sb = cpool.tile_from(bcomb_d[:], name="bcomb_sb")
            fw2t_sb = cpool.tile_from(fw2t_d[:], name="fw2t_sb")
            fb2r_sb = cpool.tile_from(fb2r_d[:], name="fb2r_sb")

            # single-sync-wait rule: pre-observe PE-read const queues with
            # dummy 1-col matmuls; DVE/ACT-read consts with dummy touches.
            dps = fps.tile([128, 1], f32, tag="dps", name="dps", bufs=1)
            nc.tensor.matmul(dps[0:128, 0:1], w1pack_sb[0:45, 0:128],
                             w1pack_sb[0:45, 0:1], start=True, stop=True)
            nc.tensor.matmul(dps[0:128, 0:1], wcpack_sb[0:128, 0:128],
                             wcpack_sb[0:128, 0:1], start=True, stop=True)
            nc.tensor.matmul(dps[0:10, 0:1], fw2t_sb[0:128, 0:10],
                             fw2t_sb[0:128, 0:1], start=True, stop=True)
            dvescr = wpool.tile([128, 44], f32, tag="dvescr", name="dvescr", bufs=1)
            nc.vector.tensor_copy(out=dvescr[:, 0:4], in_=biaspack_sb[:])
            nc.vector.tensor_copy(out=dvescr[:, 4:44], in_=fb2r_sb[:])
            actscr = wpool.tile([128, 1], f32, tag="actscr", name="actscr", bufs=1)
            nc.scalar.copy(out=actscr[:], in_=bcomb_sb[:])

            zs = []
            n_super = n_tiles // 4      # 4 tiles (2 psum pairs) per super-group
            recip_insts = []
            pair_seq = 0
            total_recips = n_super * _NSL
            recip_on_dve = set()
            if _N_DVE_RECIP > 0:
                step = max(1, total_recips // _N_DVE_RECIP)
                recip_on_dve = set(list(range(0, total_recips, step))[:_N_DVE_RECIP])

            for sl in range(_NSL):
                z = zps.tile([128, _SLICE], f32, tag="z", name=f"z{sl}")
                zs.append(z)
                for sp in range(n_super):
                    t4 = tiles[4 * sp:4 * sp + 4]
                    q = wpool.tile([128, 4 * _SLICE], f32, tag="q",
                                   name=f"q{sl}_{sp}")
                    s = wpool.tile([128, 4 * _SLICE], f16, tag="s",
                                   name=f"s{sl}_{sp}")
                    for half in range(2):   # one psum pair per half
                        cp = cps.tile([128, 2 * _SLICE], f32, tag="cp",
                                      name=f"cp{sl}_{sp}_{half}")
                        for j in range(2):
                            t = t4[2 * half + j]
                            i = 4 * sp + 2 * half + j
                            xw = xpool.tile([t["K"], _SLICE], f16, tag="xw",
                                            name=f"xw{sl}_{i}")
                            ro = int(win_offs[i]) + sl * t["K"]
                            nc.sync.dma_start(out=xw, in_=xwin[ro:ro + t["K"], :])
                            ci = cls_idx[t["cls"]]
                            nc.tensor.matmul(
                                cp[:, j * _SLICE:(j + 1) * _SLICE],
                                w1pack_sb[0:t["K"], 128 * ci:128 * ci + 128], xw,
                                start=True, stop=True)
                        ca = cls_idx[t4[2 * half]["cls"]]
                        cb = cls_idx[t4[2 * half + 1]["cls"]]
                        qh = q[:, half * 2 * _SLICE:(half + 1) * 2 * _SLICE]
                        if ca == cb:
                            nc.vector._custom_dve(
                                taylor_den, out=qh, in0=cp,
                                s0=biaspack_sb[0:128, ca:ca + 1], s1=2.0, imm2=3.0)
                        else:
                            nc.vector._custom_dve(
                                taylor_den, out=qh[:, 0:_SLICE],
                                in0=cp[:, 0:_SLICE],
                                s0=biaspack_sb[0:128, ca:ca + 1], s1=2.0, imm2=3.0)
                            nc.vector._custom_dve(
                                taylor_den, out=qh[:, _SLICE:2 * _SLICE],
                                in0=cp[:, _SLICE:2 * _SLICE],
                                s0=biaspack_sb[0:128, cb:cb + 1], s1=2.0, imm2=3.0)
                    if pair_seq in recip_on_dve:
                        ri = nc.vector._custom_dve(
                            recip_fast, out=s, in0=q,
                            s0=RC["s0"], s1=RC["s1"], imm2=RC["imm2"])
                    else:
                        ri = _act_raw(nc, s, q, AF.Reciprocal)
                    recip_insts.append(ri)
                    pair_seq += 1
                    for j4 in range(4):
                        i = 4 * sp + j4
                        nc.tensor.matmul(
                            z, wcpack_sb[0:128, 128 * i:128 * i + 128],
                            s[:, j4 * _SLICE:(j4 + 1) * _SLICE],
                            start=(i == 0), stop=(i == n_tiles - 1))
            # ---- tail: sigmoid, fc2, log_softmax (no max-sub: |logits| < 12,
            # exp cannot overflow fp32). Order all tail ACT work after the
            # last reciprocal so the ACT table sets load once each.
            last_recip = recip_insts[-1]
            hs = []
            for sl in range(_NSL):
                h = wpool.tile([128, _SLICE], f16, tag="h", name=f"h{sl}")
                si = nc.scalar.activation(h, zs[sl], AF.Sigmoid, bias=bcomb_sb[:],
                                          scale=1.0)
                add_dep_helper(si.ins, last_recip.ins, sync=False,
                               reason="keep tail ACT after recips (table sets)")
                hs.append(h)
            for sl in range(_NSL):
                ng = _SLICE // 128
                fp = fps.tile([128, 10 * ng], f32, tag="fp", name=f"fp{sl}", bufs=1)
                for g in range(ng):
                    nc.tensor.matmul(fp[:, g * 10:(g + 1) * 10],
                                     hs[sl][:, g * 128:(g + 1) * 128], fw2t_sb[:],
                                     start=True, stop=True)
                lg = wpool.tile([128, 10 * ng], f32, tag="lg", name=f"lg{sl}")
                nc.vector.tensor_tensor(out=lg, in0=fp, in1=fb2r_sb[:, 0:10 * ng],
                                        op=AluOpType.add)
                e = wpool.tile([128, 10 * ng], f32, tag="e", name=f"e{sl}")
                ei = nc.scalar.activation(e, lg, AF.Exp)
                add_dep_helper(ei.ins, last_recip.ins, sync=False,
                               reason="keep tail ACT after recips (table sets)")
                ssum = wpool.tile([128, ng], f32, tag="ss", name=f"ss{sl}")
                nc.vector.tensor_reduce(
                    ssum, e.rearrange("p (g k) -> p g k", k=10),
                    axis=mybir.AxisListType.X, op=AluOpType.add)
                lns = wpool.tile([128, ng], f32, tag="ls", name=f"ls{sl}")
                li = nc.scalar.activation(lns, ssum, AF.Ln)
                add_dep_helper(li.ins, last_recip.ins, sync=False,
                               reason="keep tail ACT after recips (table sets)")
                ot = wpool.tile([128, 10 * ng], f32, tag="ot", name=f"ot{sl}")
                for g in range(ng):
                    nc.vector.tensor_scalar(
                        out=ot[:, g * 10:(g + 1) * 10],
                        in0=lg[:, g * 10:(g + 1) * 10],
                        scalar1=lns[:, g:g + 1], scalar2=None,
                        op0=AluOpType.subtract)
                orow = sl * _SLICE
                nc.sync.dma_start(
                    out=out_d[orow:orow + _SLICE, :].rearrange(
                        "(g p) k -> p g k", p=128),
                    in_=ot.rearrange("p (g k) -> p g k", k=10))
    nc.compile()
    return nc


_PROGRAM_CACHE = {}


def kernel(x, w1, b1, w2, b2, fw1, fb1, fw2, fb2):
    global LAST_RESULTS
    wins, consts, tiles = _host_prep(x, w1, b1, w2, b2, fw1, fb1, fw2, fb2)

    if "nc" not in _PROGRAM_CACHE:
        _PROGRAM_CACHE["nc"] = _build_program(tiles, consts["cls_idx"],
                                              consts["win_offs"])
    nc = _PROGRAM_CACHE["nc"]

    shared = {k: consts[k] for k in
              ("wcpack", "w1pack", "biaspack", "bcomb", "fw2t", "fb2r")}
    in_maps = []
    for c in range(_NCORES):
        m = dict(shared)
        # per-core pre-windowed blob: per tile, per slice, [K_t, 512] blocks
        blocks = []
        for t_i, t in enumerate(tiles):
            w = wins[t_i][:, c * _PC:(c + 1) * _PC]
            for sl in range(_NSL):
                blocks.append(w[:, sl * _SLICE:(sl + 1) * _SLICE])
        m["xwin"] = np.ascontiguousarray(np.concatenate(blocks, axis=0))
        in_maps.append(m)

    from concourse.bass_utils import run_bass_kernel_spmd
    trace = bool(int(os.environ.get("BASS_KERNEL_TRACE", "0")))
    res = run_bass_kernel_spmd(nc, in_maps, core_ids=list(range(_NCORES)),
                               trace=trace)
    LAST_RESULTS = res
    return np.concatenate([r["out"] for r in res.results], axis=0)



# revision 2
# speedup vs baseline: 1.3648x; 1.3648x over previous
"""Trainium2 Bass kernel for nn_Net_39230231281866 (dense_cnn), v2.

Network: conv3x3(1->6) -> Taylor-sigmoid -> conv3x3(6->7) -> flatten
         -> fc(4032->128) -> sigmoid -> fc(128->10) -> log_softmax,
batch 8192, data-parallel over 8 NeuronCores (1024 samples/core).

v2 design (vs v1 baseline ~99us):
  * conv1: fp8 banded matmuls, TWO tiles packed concurrently in the PE
    array via row-groups (K<=48 -> tiles at base_partition 0 and 64,
    tile_position auto-derived). 35-tile cover of the 26x26 output.
  * Taylor-sigmoid replaced by ONE pass per psum pair, statically split
    between ScalarE (tanh, affine-fitted to the Taylor sigmoid) and
    VectorE (custom single-pass odd-quintic DVE op fitted likewise).
    Outputs are fp8e4 s-tiles.
  * conv2+fc1 folded into W_comb as before, but quantized fp8e4 with a
    global gain G folded out through the fc1-sigmoid, and the 36 K-chunks
    contracted as 18 DoubleRow fp8 matmuls (K=256 per instruction).
  * fc1 sigmoid computed as (sigma-0.5) via the same quintic DVE op, so
    ScalarE's exp/ln table load can hide under the end of the main loop;
    0.5 is folded into fb2. fc2 in fp16.
  * DMA: one fp8 pre-windowed input slab per core, loaded in 6 large
    chunk DMAs on the sync ring; weights on the scalar ring.
"""

import os
import numpy as np
import ml_dtypes

_B = 8192
_NCORES = 8
_PC = _B // _NCORES
_SLICE = 512
_NSL = _PC // _SLICE

_A_IN = 0.5          # conv matmul gain: cp = _A_IN * conv1(x)
_G = 64.0            # global fp8 gain for W_comb
_F8 = ml_dtypes.float8_e4m3

LAST_RESULTS = None


# ---------------- tiling ----------------

def _tiles():
    """35 rectangular tiles covering the 26x26 conv1 output, all with
    M = 6*noy*nox <= 128 and K = (noy+2)*(nox+2) <= 48 (2 fit in the PE
    rows as 64-row groups). Sorted by class so pairs share a class."""
    ts = []
    for oy0 in range(0, 24, 3):
        for (ox0, nox) in [(0, 7), (7, 7), (14, 7), (21, 5)]:
            ts.append((oy0, 3, ox0, nox))
    for (ox0, nox) in [(0, 10), (10, 10), (20, 6)]:
        ts.append((24, 2, ox0, nox))
    order = {(3, 7): 0, (3, 5): 1, (2, 10): 2, (2, 6): 3}
    ts.sort(key=lambda t: order[(t[1], t[3])])
    return ts


_TILES = _tiles()
_N_UNITS = 18            # 17 pairs + 1 single, per slice
_PAIR_CLASSES = []       # unit -> weight-matrix class index
_CLS_LIST = []
for _u in range(_N_UNITS):
    _ta = _TILES[2 * _u]
    _ca = (_ta[1], _ta[3])
    if 2 * _u + 1 < len(_TILES):
        _cb = (_TILES[2 * _u + 1][1], _TILES[2 * _u + 1][3])
    else:
        _cb = None
    if (_ca, _cb) not in _CLS_LIST:
        _CLS_LIST.append((_ca, _cb))
    _PAIR_CLASSES.append(_CLS_LIST.index((_ca, _cb)))

# unit -> engine: even units on ScalarE (tanh), odd on VectorE (quintic)
_UNIT_ON_ACT = {u: (u % 2 == 0) for u in range(_N_UNITS)}


# ---------------- fits (input-independent constants) ----------------

def _taylor_sig(x):
    t = -x
    return 1.0 / (2.0 + t * (1.0 + t * (0.5 + t * ((1.0 / 6.0) + t * (1.0 / 24.0)))))


def _fit_consts():
    # conv quintic: c5 u'^5 + c3 u'^3 + c1 u' + beta ~= taylor_sig(2u),
    # u' = u + dq, u = 0.5*(v + b1)
    u = np.linspace(-2.05, 2.05, 2051)
    w = np.exp(-0.5 * (u / 0.36) ** 2) + 3e-4
    tgt = _taylor_sig(2 * u)
    sw = np.sqrt(w)
    best = None
    for dq in np.linspace(-0.3, 0.3, 61):
        uu = u + dq
        A = np.stack([uu ** 5, uu ** 3, uu, np.ones_like(u)], 1)
        coef, *_ = np.linalg.lstsq(A * sw[:, None], tgt * sw, rcond=None)
        e = A @ coef - tgt
        L = (w * e ** 2).sum()
        if best is None or L < best[0]:
            best = (L, dq, coef)
    _, dq, (c5, c3, c1, q_beta) = best

    # tanh: a*tanh(g t + d) + b ~= taylor_sig(t), t = v + b1
    t = np.linspace(-2.6, 2.6, 2601)
    wt = np.exp(-0.5 * (t / 0.70) ** 2) + 3e-4
    tt = _taylor_sig(t)
    swt = np.sqrt(wt)
    best = None
    for g in np.linspace(0.4, 0.75, 36):
        for dd in np.linspace(-0.3, 0.3, 31):
            A = np.stack([np.tanh(g * t + dd), np.ones_like(t)], 1)
            coef, *_ = np.linalg.lstsq(A * swt[:, None], tt * swt, rcond=None)
            e = A @ coef - tt
            L = (wt * e ** 2).sum()
            if best is None or L < best[0]:
                best = (L, g, dd, coef)
    _, tg, td, (t_alpha, t_beta) = best

    # h quintic (odd): a5 y^5 + a3 y^3 + a1 y ~= sigmoid(y) - 0.5
    y = np.linspace(-1.2, 1.2, 1201)
    wy = np.exp(-0.5 * (y / 0.30) ** 2) + 1e-3
    ty = 1.0 / (1.0 + np.exp(-y)) - 0.5
    swy = np.sqrt(wy)
    A = np.stack([y ** 5, y ** 3, y], 1)
    (a5, a3, a1), *_ = np.linalg.lstsq(A * swy[:, None], ty * swy, rcond=None)
    return dict(dq=float(dq), c5=float(c5), c3=float(c3), c1=float(c1),
                q_beta=float(q_beta), tg=float(tg), td=float(td),
                t_alpha=float(t_alpha), t_beta=float(t_beta),
                a5=float(a5), a3=float(a3), a1=float(a1))


_FC = _fit_consts()


# ---------------- host prep ----------------

def _host_prep(x, w1, b1, w2, b2, fw1, fb1, fw2, fb2):
    x = np.asarray(x, np.float32)
    w1 = np.asarray(w1, np.float32); b1 = np.asarray(b1, np.float32)
    w2 = np.asarray(w2, np.float32); b2 = np.asarray(b2, np.float32)
    fw1 = np.asarray(fw1, np.float32); fb1 = np.asarray(fb1, np.float32)
    fw2 = np.asarray(fw2, np.float32); fb2 = np.asarray(fb2, np.float32)
    F = _FC

    # pair-class banded conv weights [128, n_cls*128], gain _A_IN, fp8
    w1pack = np.zeros((128, len(_CLS_LIST) * 128), np.float32)

    def _banded(cls):
        noy, nox = cls
        ky, kx = noy + 2, nox + 2
        wt = np.zeros((ky * kx, 128), np.float32)
        for oy in range(noy):
            for ox in range(nox):
                for oc in range(6):
                    m = (oy * nox + ox) * 6 + oc
                    for dy in range(3):
                        for dx in range(3):
                            wt[(oy + dy) * kx + (ox + dx), m] = \
                                _A_IN * w1[oc, 0, dy, dx]
        return wt

    for ci, (ca, cb) in enumerate(_CLS_LIST):
        wa = _banded(ca)
        w1pack[0:wa.shape[0], ci * 128:(ci + 1) * 128] = wa
        if cb is not None:
            wb = _banded(cb)
            w1pack[64:64 + wb.shape[0], ci * 128:(ci + 1) * 128] = wb

    # fold conv2 + fc1 -> Wc [128, 6*26*26], bias bcomb
    fw1r = fw1.reshape(128, 7, 24, 24)
    Wc = np.zeros((128, 6, 26, 26), np.float32)
    for dy in range(3):
        for dx in range(3):
            Wc[:, :, dy:dy + 24, dx:dx + 24] += np.einsum(
                "joyx,oi->jiyx", fw1r, w2[:, :, dy, dx], optimize=True)
    bcomb = fb1 + np.einsum("joyx,o->j", fw1r, b2)
    Wc_flat = Wc.reshape(128, 6 * 26 * 26)

    # wcpack [128, 18*256] fp8 (DoubleRow layout: unit u, j in {0,1} at
    # cols u*256 + j*128 + f), per-tile gain G*alpha; beta folds into bias
    wcpack = np.zeros((128, _N_UNITS * 256), np.float32)
    bc_eff = bcomb.copy()
    for ti, t in enumerate(_TILES):
        oy0, noy, ox0, nox = t
        M = noy * nox * 6
        unit, j = ti // 2, ti % 2
        alpha = F["t_alpha"] if _UNIT_ON_ACT[unit] else 1.0
        beta = F["t_beta"] if _UNIT_ON_ACT[unit] else F["q_beta"]
        cols = []
        for oy in range(noy):
            for ox in range(nox):
                for oc in range(6):
                    cols.append((oc * 26 + oy0 + oy) * 26 + ox0 + ox)
        Wt = Wc_flat[:, cols]                       # [128 feat, M]
        wcpack[0:M, unit * 256 + j * 128: unit * 256 + (j + 1) * 128] = \
            (_G * alpha) * Wt.T
        bc_eff += beta * Wt.sum(axis=1)

    bias_act = (F["tg"] * b1[np.arange(128) % 6] + F["td"]).astype(np.float32)
    bias_dve = (_A_IN * b1[np.arange(128) % 6] + F["dq"]).astype(np.float32)
    cvec = np.stack([np.full(128, F["c5"], np.float32),
                     np.full(128, F["a5"] / _G ** 5, np.float32)], 1)
    fb2e = fb2 + 0.5 * fw2.sum(axis=1)

    consts = dict(
        w1pack=w1pack.astype(_F8), wcpack=wcpack.astype(_F8),
        biasact=bias_act.reshape(128, 1),
        biasdve=bias_dve.reshape(128, 1),
        cvec=cvec,
        bch=(_G * bc_eff).reshape(128, 1).astype(np.float32),
        fw2t=np.ascontiguousarray(fw2.T).astype(np.float16),
        fb2r=np.tile(fb2e.reshape(1, 10), (128, 4)).astype(np.float32),
    )

    # pre-windowed fp8 input slab per core: [128, 36*512], col block
    # b = sl*18 + unit; rows 0:K_A = tile 2u window, 64:64+K_B = tile 2u+1
    x_pm = x.reshape(_B, 784).T.astype(_F8)         # [784, B]
    slabs = []
    for c in range(_NCORES):
        slab = np.zeros((128, _NSL * _N_UNITS * _SLICE), _F8)
        for sl in range(_NSL):
            s0 = c * _PC + sl * _SLICE
            for ti, t in enumerate(_TILES):
                oy0, noy, ox0, nox = t
                ky, kx = noy + 2, nox + 2
                unit, j = ti // 2, ti % 2
                rows = ((np.arange(ky)[:, None] + oy0) * 28 +
                        (np.arange(kx)[None, :] + ox0)).reshape(-1)
                cb = (sl * _N_UNITS + unit) * _SLICE
                slab[64 * j: 64 * j + ky * kx, cb:cb + _SLICE] = \
                    x_pm[rows, s0:s0 + _SLICE]
        slabs.append(slab)
    return slabs, consts


# ---------------- custom DVE op: odd quintic ----------------

def _register_sigpoly():
    import concourse.dve_ops as dve_ops
    if "SIGPOLY5_ANT" in dve_ops._SUB_OPCODE_FOR_NAME:
        return next(o for o in dve_ops.OPS if o.name == "SIGPOLY5_ANT")
    from concourse.dve_spec import (Spec, Src0, C0, C1, C2, C3, lower,
                                    _spill_c3_to_src1)
    from concourse.dve_uop import DveOpSpec

    # u = in0 + s0;  out = ((c5*u^2 + s1)*u^2 + imm2)*u   (c5 via in1)
    u = Src0 + C0
    w = u * u
    body = _spill_c3_to_src1(((C3 * w + C1) * w + C2) * u)

    def _ref(in0, in1, s0, s1, imm2):
        uu = in0.astype(np.float32) + s0
        ww = uu * uu
        c5 = np.asarray(in1, np.float32).reshape(in0.shape[0], -1)[:, :1]
        return ((c5 * ww + s1) * ww + imm2) * uu

    spec = Spec(body=body, reference=_ref)
    name = "SIGPOLY5_ANT"
    row = max(dve_ops._SUB_OPCODE_FOR_NAME.values()) + 1
    assert row < 0x20
    dve_ops._SUB_OPCODE_FOR_NAME[name] = row
    shas = {}
    for ver in ("v3", "v4"):
        tmp = DveOpSpec(name=name, opcode=row, uops=lower(spec, ver=ver),
                        rd1_en=True)
        shas[ver] = tmp.sha(ver)
    op = dve_ops.DveOp(name, spec, subdim=False, uops_sha=shas)
    dve_ops.OPS.append(op)
    dve_ops.CUSTOM_DVE_SPECS[name] = spec
    return op


def _pin_act_tables():
    """Pin Tanh -> exp_and_others, Exp/Ln -> natural_log_exp_and_others
    so the kernel costs exactly two ACT table loads."""
    import concourse.bacc as bacc
    import concourse.mybir as mybir
    if getattr(bacc, "_ant_tables_pinned", False):
        return
    orig = bacc.get_activation_tables
    AF = mybir.ActivationFunctionType

    def patched(arch):
        tabs = {k: set(v) for k, v in orig(arch).items()}
        for name, fns in tabs.items():
            if name != "exp_and_others":
                fns.discard(AF.Tanh)
            if name != "natural_log_exp_and_others":
                fns.discard(AF.Exp)
                fns.discard(AF.Ln)
        return tabs

    bacc.get_activation_tables = patched
    bacc._ant_tables_pinned = True


# ---------------- program ----------------

def _build_program():
    import concourse.bacc as bacc
    import concourse.mybir as mybir
    from concourse.tile import TileContext
    from concourse.tile_rust import add_dep_helper
    from concourse.alu_op_type import AluOpType

    f32 = mybir.dt.float32
    f16 = mybir.dt.float16
    f8 = mybir.dt.float8e4
    AF = mybir.ActivationFunctionType
    DR = mybir.MatmulPerfMode.DoubleRow
    sigpoly = _register_sigpoly()
    _pin_act_tables()
    F = _FC

    nc = bacc.Bacc()
    n_cols = _NSL * _N_UNITS * _SLICE
    xwin_d = nc.declare_dram_parameter("xwin", [128, n_cols], f8, isOutput=False)
    w1pack_d = nc.declare_dram_parameter("w1pack", [128, len(_CLS_LIST) * 128],
                                         f8, isOutput=False)
    wcpack_d = nc.declare_dram_parameter("wcpack", [128, _N_UNITS * 256], f8,
                                         isOutput=False)
    biasact_d = nc.declare_dram_parameter("biasact", [128, 1], f32, isOutput=False)
    biasdve_d = nc.declare_dram_parameter("biasdve", [128, 1], f32, isOutput=False)
    cvec_d = nc.declare_dram_parameter("cvec", [128, 2], f32, isOutput=False)
    bch_d = nc.declare_dram_parameter("bch", [128, 1], f32, isOutput=False)
    fw2t_d = nc.declare_dram_parameter("fw2t", [128, 10], f16, isOutput=False)
    fb2r_d = nc.declare_dram_parameter("fb2r", [128, 40], f32, isOutput=False)
    out_d = nc.declare_dram_parameter("out", [_PC, 10], f32, isOutput=True)

    n_chunks = 6
    ch_cols = n_cols // n_chunks        # 3072 = 6 units

    with TileContext(nc) as tc:
        with (
            tc.tile_pool(name="const", bufs=1) as cpool,
            tc.tile_pool(name="work", bufs=3) as wpool,
            tc.tile_pool(name="cps", bufs=2, space="PSUM") as cps,
            tc.tile_pool(name="zps", bufs=2, space="PSUM") as zps,
            tc.tile_pool(name="fps", bufs=1, space="PSUM") as fps,
        ):
            # weights/consts on the scalar HWDGE ring
            w1pack_sb = cpool.tile([128, len(_CLS_LIST) * 128], f8,
                                   tag="w1p", name="w1pack_sb", bufs=1)
            nc.scalar.dma_start(out=w1pack_sb, in_=w1pack_d[:])
            small = []
            for nm, par, shape, dt_ in (
                ("biasact", biasact_d, [128, 1], f32),
                ("biasdve", biasdve_d, [128, 1], f32),
                ("cvec", cvec_d, [128, 2], f32),
                ("bch", bch_d, [128, 1], f32),
                ("fw2t", fw2t_d, [128, 10], f16),
                ("fb2r", fb2r_d, [128, 40], f32),
            ):
                t = cpool.tile(shape, dt_, tag=nm, name=nm + "_sb", bufs=1)
                nc.scalar.dma_start(out=t, in_=par[:])
                small.append(t)
            biasact_sb, biasdve_sb, cvec_sb, bch_sb, fw2t_sb, fb2r_sb = small
            wcpack_sb = cpool.tile([128, _N_UNITS * 256], f8, tag="wcp",
                                   name="wcpack_sb", bufs=1)
            nc.scalar.dma_start(out=wcpack_sb, in_=wcpack_d[:])

            # xwin slab chunks on the sync ring
            xw = []
            for j in range(n_chunks):
                t = cpool.tile([128, ch_cols], f8, tag="xw", name=f"xw{j}",
                               bufs=n_chunks)
                nc.sync.dma_start(out=t, in_=xwin_d[:, j * ch_cols:(j + 1) * ch_cols])
                xw.append(t)

            # pre-observe const queues (single-sync-wait rule)
            dps = fps.tile([128, 1], f32, tag="dps", name="dps", bufs=1)
            nc.tensor.matmul(dps[0:128, 0:1], w1pack_sb[0:45, 0:128],
                             w1pack_sb[0:45, 0:1], start=True, stop=True)
            nc.tensor.matmul(dps[0:128, 0:1], wcpack_sb[0:128, 0:128],
                             wcpack_sb[0:128, 0:1], start=True, stop=True)
            nc.tensor.matmul(dps[0:10, 0:1], fw2t_sb[0:128, 0:10],
                             fw2t_sb[0:128, 0:1], start=True, stop=True)
            dvescr = wpool.tile([128, 44], f32, tag="dvescr", name="dvescr",
                                bufs=1)
            nc.vector.tensor_copy(out=dvescr[:, 0:1], in_=biasdve_sb[:])
            nc.vector.tensor_copy(out=dvescr[:, 1:3], in_=cvec_sb[:])
            nc.vector.tensor_copy(out=dvescr[:, 3:4], in_=bch_sb[:])
            nc.vector.tensor_copy(out=dvescr[:, 4:44], in_=fb2r_sb[:])
            actscr = wpool.tile([128, 1], f32, tag="actscr", name="actscr",
                                bufs=1)
            nc.scalar.copy(out=actscr[:], in_=biasact_sb[:])

            tanh_insts = []
            zs = []
            for sl in range(_NSL):
                z = zps.tile([128, _SLICE], f32, tag="z", name=f"z{sl}")
                zs.append(z)
                for u in range(_N_UNITS):
                    ci = _PAIR_CLASSES[u]
                    cb = (sl * _N_UNITS + u) * _SLICE
                    ch, lo = cb // ch_cols, cb % ch_cols
                    ta = _TILES[2 * u]
                    ka = (ta[1] + 2) * (ta[3] + 2)
                    single = (2 * u + 1 >= len(_TILES))
                    ncol = _SLICE if single else 2 * _SLICE
                    cp = cps.tile([128, 2 * _SLICE], f32, tag="cp",
                                  name=f"cp{sl}_{u}")
                    nc.tensor.matmul(
                        cp[:, 0:_SLICE],
                        w1pack_sb[0:ka, ci * 128:ci * 128 + 128],
                        xw[ch][0:ka, lo:lo + _SLICE], start=True, stop=True)
                    if not single:
                        tb = _TILES[2 * u + 1]
                        kb = (tb[1] + 2) * (tb[3] + 2)
                        nc.tensor.matmul(
                            cp[:, _SLICE:2 * _SLICE],
                            w1pack_sb[64:64 + kb, ci * 128:ci * 128 + 128],
                            xw[ch][64:64 + kb, lo:lo + _SLICE],
                            start=True, stop=True)
                    s = wpool.tile([128, 2 * _SLICE], f8, tag="s",
                                   name=f"s{sl}_{u}")
                    if _UNIT_ON_ACT[u]:
                        ti_ = nc.scalar.activation(
                            s[:, 0:ncol], cp[:, 0:ncol], AF.Tanh,
                            bias=biasact_sb[:], scale=F["tg"] / _A_IN)
                        tanh_insts.append(ti_)
                    else:
                        nc.vector._custom_dve(
                            sigpoly, out=s[:, 0:ncol], in0=cp[:, 0:ncol],
                            in1=cvec_sb[:, 0:1], s0=biasdve_sb[:],
                            s1=F["c3"], imm2=F["c1"])
                    if single:
                        nc.tensor.matmul(
                            z, wcpack_sb[:, u * 256:u * 256 + 128],
                            s[:, 0:_SLICE], start=(u == 0),
                            stop=(u == _N_UNITS - 1))
                    else:
                        nc.tensor.matmul(
                            z,
                            wcpack_sb[:, u * 256:(u + 1) * 256].rearrange(
                                "p (j f) -> p j f", j=2),
                            s.rearrange("p (j n) -> p j n", j=2),
                            start=(u == 0), stop=(u == _N_UNITS - 1),
                            perf_mode=DR)

            # ---- tail: h = (sigma-0.5) via quintic, fc2, log_softmax ----
            last_tanh = tanh_insts[-1]
            for sl in range(_NSL):
                hp = wpool.tile([128, _SLICE], f16, tag="h", name=f"h{sl}")
                nc.vector._custom_dve(
                    sigpoly, out=hp, in0=zs[sl], in1=cvec_sb[:, 1:2],
                    s0=bch_sb[:], s1=F["a3"] / _G ** 3, imm2=F["a1"] / _G)
                ng = _SLICE // 128
                fp = fps.tile([128, 10 * ng], f32, tag="fp", name=f"fp{sl}",
                              bufs=1)
                for g in range(ng):
                    nc.tensor.matmul(fp[:, g * 10:(g + 1) * 10],
                                     hp[:, g * 128:(g + 1) * 128],
                                     fw2t_sb[:], start=True, stop=True)
                lg = wpool.tile([128, 10 * ng], f32, tag="lg", name=f"lg{sl}")
                nc.vector.tensor_tensor(out=lg, in0=fp, in1=fb2r_sb[:, 0:10 * ng],
                                        op=AluOpType.add)
                e = wpool.tile([128, 10 * ng], f32, tag="e", name=f"e{sl}")
                ei = nc.scalar.activation(e, lg, AF.Exp)
                add_dep_helper(ei.ins, last_tanh.ins, sync=False,
                               reason="exp after last tanh (table sets)")
                ssum = wpool.tile([128, ng], f32, tag="ss", name=f"ss{sl}")
                nc.vector.tensor_reduce(
                    ssum, e.rearrange("p (g k) -> p g k", k=10),
                    axis=mybir.AxisListType.X, op=AluOpType.add)
                lns = wpool.tile([128, ng], f32, tag="ls", name=f"ls{sl}")
                li = nc.scalar.activation(lns, ssum, AF.Ln)
                add_dep_helper(li.ins, last_tanh.ins, sync=False,
                               reason="ln after last tanh (table sets)")
                ot = wpool.tile([128, 10 * ng], f32, tag="ot", name=f"ot{sl}")
                for g in range(ng):
                    nc.vector.tensor_scalar(
                        out=ot[:, g * 10:(g + 1) * 10],
                        in0=lg[:, g * 10:(g + 1) * 10],
                        scalar1=lns[:, g:g + 1], scalar2=None,
                        op0=AluOpType.subtract)
                orow = sl * _SLICE
                nc.sync.dma_start(
                    out=out_d[orow:orow + _SLICE, :].rearrange(
                        "(g p) k -> p g k", p=128),
                    in_=ot.rearrange("p (g k) -> p g k", k=10))
    nc.compile()
    return nc


_PROGRAM_CACHE = {}


def kernel(x, w1, b1, w2, b2, fw1, fb1, fw2, fb2):
    global LAST_RESULTS
    slabs, consts = _host_prep(x, w1, b1, w2, b2, fw1, fb1, fw2, fb2)

    if "nc" not in _PROGRAM_CACHE:
        _PROGRAM_CACHE["nc"] = _build_program()
    nc = _PROGRAM_CACHE["nc"]

    in_maps = []
    for c in range(_NCORES):
        m = dict(consts)
        m["xwin"] = slabs[c]
        in_maps.append(m)

    from concourse.bass_utils import run_bass_kernel_spmd
    trace = bool(int(os.environ.get("BASS_KERNEL_TRACE", "0")))
    res = run_bass_kernel_spmd(nc, in_maps, core_ids=list(range(_NCORES)),
                               trace=trace)
    LAST_RESULTS = res
    return np.concatenate([r["out"] for r in res.results], axis=0)


# revision 7
# speedup vs baseline: 1.4105x; 1.0335x over previous
"""Trainium2 Bass kernel for nn_Net_39230231281866 (dense_cnn), v2.

Network: conv3x3(1->6) -> Taylor-sigmoid -> conv3x3(6->7) -> flatten
         -> fc(4032->128) -> sigmoid -> fc(128->10) -> log_softmax,
batch 8192, data-parallel over 8 NeuronCores (1024 samples/core).

v2 design (vs v1 baseline ~99us):
  * conv1: fp8 banded matmuls, TWO tiles packed concurrently in the PE
    array via row-groups (K<=48 -> tiles at base_partition 0 and 64,
    tile_position auto-derived). 35-tile cover of the 26x26 output.
  * Taylor-sigmoid replaced by ONE pass per psum pair, statically split
    between ScalarE (tanh, affine-fitted to the Taylor sigmoid) and
    VectorE (custom single-pass odd-quintic DVE op fitted likewise).
    Outputs are fp8e4 s-tiles.
  * conv2+fc1 folded into W_comb as before, but quantized fp8e4 with a
    global gain G folded out through the fc1-sigmoid, and the 36 K-chunks
    contracted as 18 DoubleRow fp8 matmuls (K=256 per instruction).
  * fc1 sigmoid computed as (sigma-0.5) via the same quintic DVE op, so
    ScalarE's exp/ln table load can hide under the end of the main loop;
    0.5 is folded into fb2. fc2 in fp16.
  * DMA: one fp8 pre-windowed input slab per core, loaded in 6 large
    chunk DMAs on the sync ring; weights on the scalar ring.
"""

import os
import numpy as np
import ml_dtypes

_B = 8192
_NCORES = 8
_PC = _B // _NCORES
_SLICE = 512
_NSL = _PC // _SLICE

_A_IN = 0.5          # conv matmul gain: cp = _A_IN * conv1(x)
_G = 64.0            # global fp8 gain for W_comb
_F8 = ml_dtypes.float8_e4m3

LAST_RESULTS = None


# ---------------- tiling ----------------

def _tiles():
    """35 rectangular tiles covering the 26x26 conv1 output, all with
    M = 6*noy*nox <= 128 and K = (noy+2)*(nox+2) <= 48 (2 fit in the PE
    rows as 64-row groups). Sorted by class so pairs share a class."""
    ts = []
    for oy0 in range(0, 24, 3):
        for (ox0, nox) in [(0, 7), (7, 7), (14, 7), (21, 5)]:
            ts.append((oy0, 3, ox0, nox))
    for (ox0, nox) in [(0, 10), (10, 10), (20, 6)]:
        ts.append((24, 2, ox0, nox))
    order = {(3, 7): 0, (3, 5): 1, (2, 10): 2, (2, 6): 3}
    ts.sort(key=lambda t: order[(t[1], t[3])])
    return ts


_TILES = _tiles()
_N_UNITS = 18            # 17 pairs + 1 single, per slice
_PAIR_CLASSES = []       # unit -> weight-matrix class index
_CLS_LIST = []
for _u in range(_N_UNITS):
    _ta = _TILES[2 * _u]
    _ca = (_ta[1], _ta[3])
    if 2 * _u + 1 < len(_TILES):
        _cb = (_TILES[2 * _u + 1][1], _TILES[2 * _u + 1][3])
    else:
        _cb = None
    if (_ca, _cb) not in _CLS_LIST:
        _CLS_LIST.append((_ca, _cb))
    _PAIR_CLASSES.append(_CLS_LIST.index((_ca, _cb)))

# unit -> engine: even units on ScalarE (tanh), odd on VectorE (quintic)
_UNIT_ON_ACT = {u: (u % 2 == 0) for u in range(_N_UNITS)}


# ---------------- fits (input-independent constants) ----------------

def _taylor_sig(x):
    t = -x
    return 1.0 / (2.0 + t * (1.0 + t * (0.5 + t * ((1.0 / 6.0) + t * (1.0 / 24.0)))))


def _fit_consts():
    # conv quintic: c5 u'^5 + c3 u'^3 + c1 u' + beta ~= taylor_sig(2u),
    # u' = u + dq, u = 0.5*(v + b1)
    u = np.linspace(-2.05, 2.05, 2051)
    w = np.exp(-0.5 * (u / 0.36) ** 2) + 3e-4
    tgt = _taylor_sig(2 * u)
    sw = np.sqrt(w)
    best = None
    for dq in np.linspace(-0.3, 0.3, 61):
        uu = u + dq
        A = np.stack([uu ** 5, uu ** 3, uu, np.ones_like(u)], 1)
        coef, *_ = np.linalg.lstsq(A * sw[:, None], tgt * sw, rcond=None)
        e = A @ coef - tgt
        L = (w * e ** 2).sum()
        if best is None or L < best[0]:
            best = (L, dq, coef)
    _, dq, (c5, c3, c1, q_beta) = best

    # tanh: a*tanh(g t + d) + b ~= taylor_sig(t), t = v + b1
    t = np.linspace(-2.6, 2.6, 2601)
    wt = np.exp(-0.5 * (t / 0.70) ** 2) + 3e-4
    tt = _taylor_sig(t)
    swt = np.sqrt(wt)
    best = None
    for g in np.linspace(0.4, 0.75, 36):
        for dd in np.linspace(-0.3, 0.3, 31):
            A = np.stack([np.tanh(g * t + dd), np.ones_like(t)], 1)
            coef, *_ = np.linalg.lstsq(A * swt[:, None], tt * swt, rcond=None)
            e = A @ coef - tt
            L = (wt * e ** 2).sum()
            if best is None or L < best[0]:
                best = (L, g, dd, coef)
    _, tg, td, (t_alpha, t_beta) = best

    # h quintic (odd): a5 y^5 + a3 y^3 + a1 y ~= sigmoid(y) - 0.5
    y = np.linspace(-1.2, 1.2, 1201)
    wy = np.exp(-0.5 * (y / 0.30) ** 2) + 1e-3
    ty = 1.0 / (1.0 + np.exp(-y)) - 0.5
    swy = np.sqrt(wy)
    A = np.stack([y ** 5, y ** 3, y], 1)
    (a5, a3, a1), *_ = np.linalg.lstsq(A * swy[:, None], ty * swy, rcond=None)
    return dict(dq=float(dq), c5=float(c5), c3=float(c3), c1=float(c1),
                q_beta=float(q_beta), tg=float(tg), td=float(td),
                t_alpha=float(t_alpha), t_beta=float(t_beta),
                a5=float(a5), a3=float(a3), a1=float(a1))


_FC = _fit_consts()


# ---------------- host prep ----------------

def _kh(cls):
    return ((cls[0] + 2) * (cls[1] + 2) + 1) // 2


# quads of 4 tiles -> 4 concurrent 32-row-group DoubleRow conv matmuls
_N_QUADS = (len(_TILES) + 3) // 4
_QUAD_CLASSES = []
_QCLS_LIST = []
for _q in range(_N_QUADS):
    _key = tuple((_TILES[i][1], _TILES[i][3]) if i < len(_TILES) else None
                 for i in range(4 * _q, 4 * _q + 4))
    if _key not in _QCLS_LIST:
        _QCLS_LIST.append(_key)
    _QUAD_CLASSES.append(_QCLS_LIST.index(_key))


def _host_prep(x, w1, b1, w2, b2, fw1, fb1, fw2, fb2):
    x = np.asarray(x, np.float32)
    w1 = np.asarray(w1, np.float32); b1 = np.asarray(b1, np.float32)
    w2 = np.asarray(w2, np.float32); b2 = np.asarray(b2, np.float32)
    fw1 = np.asarray(fw1, np.float32); fb1 = np.asarray(fb1, np.float32)
    fw2 = np.asarray(fw2, np.float32); fb2 = np.asarray(fb2, np.float32)
    F = _FC

    def _banded(cls):
        noy, nox = cls
        ky, kx = noy + 2, nox + 2
        wt = np.zeros((ky * kx, 128), np.float32)
        for oy in range(noy):
            for ox in range(nox):
                for oc in range(6):
                    m = (oy * nox + ox) * 6 + oc
                    for dy in range(3):
                        for dx in range(3):
                            wt[(oy + dy) * kx + (ox + dx), m] = \
                                _A_IN * w1[oc, 0, dy, dx]
        return wt

    # quad-class DoubleRow conv weights: tile i of a quad at partitions
    # 32i:32i+Kh, cols qc*256 + j*128 + f holds wt[j*Kh + p, f]
    w1pack = np.zeros((128, len(_QCLS_LIST) * 256), np.float32)
    for qc, key in enumerate(_QCLS_LIST):
        for i, cls in enumerate(key):
            if cls is None:
                continue
            wt = _banded(cls)
            K = wt.shape[0]
            kh = _kh(cls)
            wsplit = np.zeros((2 * kh, 128), np.float32)
            wsplit[:K] = wt
            for j in range(2):
                w1pack[32 * i:32 * i + kh,
                       qc * 256 + j * 128: qc * 256 + (j + 1) * 128] = \
                    wsplit[j * kh:(j + 1) * kh]

    # fold conv2 + fc1 -> Wc [128, 6*26*26], bias bcomb
    fw1r = fw1.reshape(128, 7, 24, 24)
    Wc = np.zeros((128, 6, 26, 26), np.float32)
    for dy in range(3):
        for dx in range(3):
            Wc[:, :, dy:dy + 24, dx:dx + 24] += np.einsum(
                "joyx,oi->jiyx", fw1r, w2[:, :, dy, dx], optimize=True)
    bcomb = fb1 + np.einsum("joyx,o->j", fw1r, b2)
    Wc_flat = Wc.reshape(128, 6 * 26 * 26)

    # wcpack [128, 18*256] fp8 (DoubleRow layout: unit u, j in {0,1} at
    # cols u*256 + j*128 + f), per-tile gain G*alpha; beta folds into bias
    wcpack = np.zeros((128, _N_UNITS * 256), np.float32)
    bc_eff = bcomb.copy()
    for ti, t in enumerate(_TILES):
        oy0, noy, ox0, nox = t
        M = noy * nox * 6
        unit, j = ti // 2, ti % 2
        alpha = F["t_alpha"] if _UNIT_ON_ACT[unit] else 1.0
        beta = F["t_beta"] if _UNIT_ON_ACT[unit] else F["q_beta"]
        cols = []
        for oy in range(noy):
            for ox in range(nox):
                for oc in range(6):
                    cols.append((oc * 26 + oy0 + oy) * 26 + ox0 + ox)
        Wt = Wc_flat[:, cols]                       # [128 feat, M]
        wcpack[0:M, unit * 256 + j * 128: unit * 256 + (j + 1) * 128] = \
            (_G * alpha) * Wt.T
        bc_eff += beta * Wt.sum(axis=1)

    bias_act = (F["tg"] * b1[np.arange(128) % 6] + F["td"]).astype(np.float32)
    bias_dve = (_A_IN * b1[np.arange(128) % 6] + F["dq"]).astype(np.float32)
    cvec = np.stack([np.full(128, F["c5"], np.float32),
                     np.full(128, F["a5"] / _G ** 5, np.float32)], 1)
    fb2e = fb2 + 0.5 * fw2.sum(axis=1)

    consts = dict(
        w1pack=w1pack.astype(_F8), wcpack=wcpack.astype(_F8),
        biasact=bias_act.reshape(128, 1),
        biasdve=bias_dve.reshape(128, 1),
        cvec=cvec,
        bch=(_G * bc_eff).reshape(128, 1).astype(np.float32),
        fw2t=np.ascontiguousarray(fw2.T).astype(np.float16),
        fb2r=np.tile(fb2e.reshape(1, 10), (128, 4)).astype(np.float32),
    )

    # pre-windowed fp8 input slab per core: [128, nsl*nquads*1024].
    # Quad block (sl, q) at cols (sl*nq+q)*1024; tile i of the quad at
    # partitions 32i:32i+Kh with its split window rows as two 512-col
    # j-blocks (DoubleRow layout).
    x_pm = x.reshape(_B, 784).T.astype(_F8)         # [784, B]
    x_pm_pad = np.zeros((785, _B), _F8)             # row 784 stays zero pad
    x_pm_pad[:784] = x_pm
    slabs = []
    for c in range(_NCORES):
        slab = np.zeros((128, _NSL * _N_QUADS * 2 * _SLICE), _F8)
        for sl in range(_NSL):
            s0 = c * _PC + sl * _SLICE
            for ti, t in enumerate(_TILES):
                oy0, noy, ox0, nox = t
                ky, kx = noy + 2, nox + 2
                q, i = ti // 4, ti % 4
                kh = _kh((noy, nox))
                rows = ((np.arange(ky)[:, None] + oy0) * 28 +
                        (np.arange(kx)[None, :] + ox0)).reshape(-1)
                rows = np.concatenate(
                    [rows, np.full(2 * kh - ky * kx, 784, np.int64)])
                cb = (sl * _N_QUADS + q) * 2 * _SLICE
                for j in range(2):
                    slab[32 * i:32 * i + kh,
                         cb + j * _SLICE: cb + (j + 1) * _SLICE] = \
                        x_pm_pad[rows[j * kh:(j + 1) * kh], s0:s0 + _SLICE]
        slabs.append(slab)
    return slabs, consts


# ---------------- custom DVE op: odd quintic ----------------

def _register_sigpoly():
    import concourse.dve_ops as dve_ops
    if "SIGPOLY5_ANT" in dve_ops._SUB_OPCODE_FOR_NAME:
        return next(o for o in dve_ops.OPS if o.name == "SIGPOLY5_ANT")
    from concourse.dve_spec import (Spec, Src0, C0, C1, C2, C3, lower,
                                    _spill_c3_to_src1)
    from concourse.dve_uop import DveOpSpec

    # u = in0 + s0;  out = ((c5*u^2 + s1)*u^2 + imm2)*u   (c5 via in1)
    u = Src0 + C0
    w = u * u
    body = _spill_c3_to_src1(((C3 * w + C1) * w + C2) * u)

    def _ref(in0, in1, s0, s1, imm2):
        uu = in0.astype(np.float32) + s0
        ww = uu * uu
        c5 = np.asarray(in1, np.float32).reshape(in0.shape[0], -1)[:, :1]
        return ((c5 * ww + s1) * ww + imm2) * uu

    spec = Spec(body=body, reference=_ref)
    name = "SIGPOLY5_ANT"
    row = max(dve_ops._SUB_OPCODE_FOR_NAME.values()) + 1
    assert row < 0x20
    dve_ops._SUB_OPCODE_FOR_NAME[name] = row
    shas = {}
    for ver in ("v3", "v4"):
        tmp = DveOpSpec(name=name, opcode=row, uops=lower(spec, ver=ver),
                        rd1_en=True)
        shas[ver] = tmp.sha(ver)
    op = dve_ops.DveOp(name, spec, subdim=False, uops_sha=shas)
    dve_ops.OPS.append(op)
    dve_ops.CUSTOM_DVE_SPECS[name] = spec
    return op


def _pin_act_tables():
    """Pin Tanh -> exp_and_others, Exp/Ln -> natural_log_exp_and_others
    so the kernel costs exactly two ACT table loads."""
    import concourse.bacc as bacc
    import concourse.mybir as mybir
    if getattr(bacc, "_ant_tables_pinned", False):
        return
    orig = bacc.get_activation_tables
    AF = mybir.ActivationFunctionType

    def patched(arch):
        tabs = {k: set(v) for k, v in orig(arch).items()}
        for name, fns in tabs.items():
            if name != "exp_and_others":
                fns.discard(AF.Tanh)
            if name != "natural_log_exp_and_others":
                fns.discard(AF.Exp)
                fns.discard(AF.Ln)
        return tabs

    bacc.get_activation_tables = patched
    bacc._ant_tables_pinned = True


# ---------------- program ----------------

def _build_program():
    import concourse.bacc as bacc
    import concourse.mybir as mybir
    from concourse.tile import TileContext
    from concourse.tile_rust import add_dep_helper
    from concourse.alu_op_type import AluOpType

    f32 = mybir.dt.float32
    f16 = mybir.dt.float16
    f8 = mybir.dt.float8e4
    AF = mybir.ActivationFunctionType
    DR = mybir.MatmulPerfMode.DoubleRow
    sigpoly = _register_sigpoly()
    _pin_act_tables()
    F = _FC

    nc = bacc.Bacc()
    n_cols = _NSL * _N_QUADS * 2 * _SLICE
    xwin_d = nc.declare_dram_parameter("xwin", [128, n_cols], f8, isOutput=False)
    w1pack_d = nc.declare_dram_parameter("w1pack", [128, len(_QCLS_LIST) * 256],
                                         f8, isOutput=False)
    wcpack_d = nc.declare_dram_parameter("wcpack", [128, _N_UNITS * 256], f8,
                                         isOutput=False)
    biasact_d = nc.declare_dram_parameter("biasact", [128, 1], f32, isOutput=False)
    biasdve_d = nc.declare_dram_parameter("biasdve", [128, 1], f32, isOutput=False)
    cvec_d = nc.declare_dram_parameter("cvec", [128, 2], f32, isOutput=False)
    bch_d = nc.declare_dram_parameter("bch", [128, 1], f32, isOutput=False)
    fw2t_d = nc.declare_dram_parameter("fw2t", [128, 10], f16, isOutput=False)
    fb2r_d = nc.declare_dram_parameter("fb2r", [128, 40], f32, isOutput=False)
    out_d = nc.declare_dram_parameter("out", [_PC, 10], f32, isOutput=True)

    n_chunks = 6
    ch_cols = n_cols // n_chunks        # 3072 = 6 units

    with TileContext(nc) as tc:
        with (
            tc.tile_pool(name="const", bufs=1) as cpool,
            tc.tile_pool(name="work", bufs=3) as wpool,
            tc.tile_pool(name="cps", bufs=3, space="PSUM") as cps,
            tc.tile_pool(name="zps", bufs=1, space="PSUM") as zps,
            tc.tile_pool(name="fps", bufs=1, space="PSUM") as fps,
        ):
            # weights/consts on the scalar HWDGE ring (w1pack/wcpack first)
            w1pack_sb = cpool.tile([128, len(_QCLS_LIST) * 256], f8,
                                   tag="w1p", name="w1pack_sb", bufs=1)
            nc.scalar.dma_start(out=w1pack_sb, in_=w1pack_d[:])
            wcpack_sb = cpool.tile([128, _N_UNITS * 256], f8, tag="wcp",
                                   name="wcpack_sb", bufs=1)
            nc.scalar.dma_start(out=wcpack_sb, in_=wcpack_d[:])
            small = []
            for nm, par, shape, dt_ in (
                ("biasact", biasact_d, [128, 1], f32),
                ("biasdve", biasdve_d, [128, 1], f32),
                ("cvec", cvec_d, [128, 2], f32),
                ("bch", bch_d, [128, 1], f32),
                ("fw2t", fw2t_d, [128, 10], f16),
                ("fb2r", fb2r_d, [128, 40], f32),
            ):
                t = cpool.tile(shape, dt_, tag=nm, name=nm + "_sb", bufs=1)
                nc.scalar.dma_start(out=t, in_=par[:])
                small.append(t)
            biasact_sb, biasdve_sb, cvec_sb, bch_sb, fw2t_sb, fb2r_sb = small

            # xwin slab chunks on the sync ring
            xw = []
            for j in range(n_chunks):
                t = cpool.tile([128, ch_cols], f8, tag="xw", name=f"xw{j}",
                               bufs=n_chunks)
                nc.sync.dma_start(out=t, in_=xwin_d[:, j * ch_cols:(j + 1) * ch_cols])
                xw.append(t)

            tanh_insts = []
            zs = []
            for sl in range(_NSL):
                z = zps.tile([128, _SLICE], f32, tag="z", name=f"z{sl}")
                zs.append(z)
                for q in range(_N_QUADS):
                    qc = _QUAD_CLASSES[q]
                    cb = (sl * _N_QUADS + q) * 2 * _SLICE
                    ch, lo = cb // ch_cols, cb % ch_cols
                    ntiles = min(4, len(_TILES) - 4 * q)
                    nhalf = (ntiles + 1) // 2
                    cpt = [cps.tile([128, 2 * _SLICE], f32, tag="cp",
                                    name=f"cp{sl}_{q}_{h}") for h in range(nhalf)]
                    for i in range(ntiles):
                        t = _TILES[4 * q + i]
                        kh = _kh((t[1], t[3]))
                        nc.tensor.matmul(
                            cpt[i // 2][:, (i % 2) * _SLICE:(i % 2 + 1) * _SLICE],
                            w1pack_sb[32 * i:32 * i + kh,
                                      qc * 256:(qc + 1) * 256].rearrange(
                                          "p (j f) -> p j f", j=2),
                            xw[ch][32 * i:32 * i + kh,
                                   lo:lo + 2 * _SLICE].rearrange(
                                       "p (j n) -> p j n", j=2),
                            start=True, stop=True, perf_mode=DR,
                            tile_position=(32 * i, 0))
                    for h in range(nhalf):
                        u = 2 * q + h
                        cp = cpt[h]
                        single = (2 * u + 1 >= len(_TILES))
                        ncol = _SLICE if single else 2 * _SLICE
                        s = wpool.tile([128, 2 * _SLICE], f8, tag="s",
                                       name=f"s{sl}_{u}")
                        if _UNIT_ON_ACT[u]:
                            ti_ = nc.scalar.activation(
                                s[:, 0:ncol], cp[:, 0:ncol], AF.Tanh,
                                bias=biasact_sb[:], scale=F["tg"] / _A_IN)
                            tanh_insts.append(ti_)
                        else:
                            nc.vector._custom_dve(
                                sigpoly, out=s[:, 0:ncol], in0=cp[:, 0:ncol],
                                in1=cvec_sb[:, 0:1], s0=biasdve_sb[:],
                                s1=F["c3"], imm2=F["c1"])
                        if single:
                            nc.tensor.matmul(
                                z, wcpack_sb[:, u * 256:u * 256 + 128],
                                s[:, 0:_SLICE], start=(u == 0),
                                stop=(u == _N_UNITS - 1))
                        else:
                            nc.tensor.matmul(
                                z,
                                wcpack_sb[:, u * 256:(u + 1) * 256].rearrange(
                                    "p (j f) -> p j f", j=2),
                                s.rearrange("p (j n) -> p j n", j=2),
                                start=(u == 0), stop=(u == _N_UNITS - 1),
                                perf_mode=DR)

            # ---- tail: h = (sigma-0.5) via quintic, fc2, log_softmax ----
            last_tanh = tanh_insts[-1]
            for sl in range(_NSL):
                hp = wpool.tile([128, _SLICE], f16, tag="h", name=f"h{sl}")
                nc.vector._custom_dve(
                    sigpoly, out=hp, in0=zs[sl], in1=cvec_sb[:, 1:2],
                    s0=bch_sb[:], s1=F["a3"] / _G ** 3, imm2=F["a1"] / _G)
                ng = _SLICE // 128
                fp = fps.tile([128, 10 * ng], f32, tag="fp", name=f"fp{sl}",
                              bufs=1)
                for g in range(ng):
                    nc.tensor.matmul(fp[:, g * 10:(g + 1) * 10],
                                     hp[:, g * 128:(g + 1) * 128],
                                     fw2t_sb[:], start=True, stop=True)
                lg = wpool.tile([128, 10 * ng], f32, tag="lg", name=f"lg{sl}")
                nc.vector.tensor_tensor(out=lg, in0=fp, in1=fb2r_sb[:, 0:10 * ng],
                                        op=AluOpType.add)
                e = wpool.tile([128, 10 * ng], f32, tag="e", name=f"e{sl}")
                ei = nc.scalar.activation(e, lg, AF.Exp)
                add_dep_helper(ei.ins, last_tanh.ins, sync=False,
                               reason="exp after last tanh (table sets)")
                ssum = wpool.tile([128, ng], f32, tag="ss", name=f"ss{sl}")
                nc.vector.tensor_reduce(
                    ssum, e.rearrange("p (g k) -> p g k", k=10),
                    axis=mybir.AxisListType.X, op=AluOpType.add)
                lns = wpool.tile([128, ng], f32, tag="ls", name=f"ls{sl}")
                li = nc.scalar.activation(lns, ssum, AF.Ln)
                add_dep_helper(li.ins, last_tanh.ins, sync=False,
                               reason="ln after last tanh (table sets)")
                ot = wpool.tile([128, 10 * ng], f32, tag="ot", name=f"ot{sl}")
                for g in range(ng):
                    nc.vector.tensor_scalar(
                        out=ot[:, g * 10:(g + 1) * 10],
                        in0=lg[:, g * 10:(g + 1) * 10],
                        scalar1=lns[:, g:g + 1], scalar2=None,
                        op0=AluOpType.subtract)
                orow = sl * _SLICE
                nc.sync.dma_start(
                    out=out_d[orow:orow + _SLICE, :].rearrange(
                        "(g p) k -> p g k", p=128),
                    in_=ot.rearrange("p (g k) -> p g k", k=10))
    nc.compile()
    return nc


_PROGRAM_CACHE = {}


def kernel(x, w1, b1, w2, b2, fw1, fb1, fw2, fb2):
    global LAST_RESULTS
    slabs, consts = _host_prep(x, w1, b1, w2, b2, fw1, fb1, fw2, fb2)

    if "nc" not in _PROGRAM_CACHE:
        _PROGRAM_CACHE["nc"] = _build_program()
    nc = _PROGRAM_CACHE["nc"]

    in_maps = []
    for c in range(_NCORES):
        m = dict(consts)
        m["xwin"] = slabs[c]
        in_maps.append(m)

    from concourse.bass_utils import run_bass_kernel_spmd
    trace = bool(int(os.environ.get("BASS_KERNEL_TRACE", "0")))
    res = run_bass_kernel_spmd(nc, in_maps, core_ids=list(range(_NCORES)),
                               trace=trace)
    LAST_RESULTS = res
    return np.concatenate([r["out"] for r in res.results], axis=0)


# revision 12
# speedup vs baseline: 1.6862x; 1.1954x over previous
"""Trainium2 Bass kernel for nn_Net_39230231281866 (dense_cnn), v2.

Network: conv3x3(1->6) -> Taylor-sigmoid -> conv3x3(6->7) -> flatten
         -> fc(4032->128) -> sigmoid -> fc(128->10) -> log_softmax,
batch 8192, data-parallel over 8 NeuronCores (1024 samples/core).

v2 design (vs v1 baseline ~99us):
  * conv1: fp8 banded matmuls, TWO tiles packed concurrently in the PE
    array via row-groups (K<=48 -> tiles at base_partition 0 and 64,
    tile_position auto-derived). 35-tile cover of the 26x26 output.
  * Taylor-sigmoid replaced by ONE pass per psum pair, statically split
    between ScalarE (tanh, affine-fitted to the Taylor sigmoid) and
    VectorE (custom single-pass odd-quintic DVE op fitted likewise).
    Outputs are fp8e4 s-tiles.
  * conv2+fc1 folded into W_comb as before, but quantized fp8e4 with a
    global gain G folded out through the fc1-sigmoid, and the 36 K-chunks
    contracted as 18 DoubleRow fp8 matmuls (K=256 per instruction).
  * fc1 sigmoid computed as (sigma-0.5) via the same quintic DVE op, so
    ScalarE's exp/ln table load can hide under the end of the main loop;
    0.5 is folded into fb2. fc2 in fp16.
  * DMA: one fp8 pre-windowed input slab per core, loaded in 6 large
    chunk DMAs on the sync ring; weights on the scalar ring.
"""

import os
import numpy as np
import ml_dtypes

_B = 8192
_NCORES = 8
_PC = _B // _NCORES
_SLICE = 512
_NSL = _PC // _SLICE

_A_IN = 0.5          # conv matmul gain: cp = _A_IN * conv1(x)
_G = 64.0            # global fp8 gain for W_comb
_F8 = ml_dtypes.float8_e4m3

LAST_RESULTS = None


# ---------------- tiling ----------------

def _tiles():
    """35 rectangular tiles covering the 26x26 conv1 output, all with
    M = 6*noy*nox <= 128 and K = (noy+2)*(nox+2) <= 48 (2 fit in the PE
    rows as 64-row groups). Sorted by class so pairs share a class."""
    ts = []
    for oy0 in range(0, 24, 3):
        for (ox0, nox) in [(0, 7), (7, 7), (14, 7), (21, 5)]:
            ts.append((oy0, 3, ox0, nox))
    for (ox0, nox) in [(0, 10), (10, 10), (20, 6)]:
        ts.append((24, 2, ox0, nox))
    order = {(3, 7): 0, (3, 5): 1, (2, 10): 2, (2, 6): 3}
    ts.sort(key=lambda t: order[(t[1], t[3])])
    return ts


_TILES = _tiles()
_N_UNITS = 18            # 17 pairs + 1 single, per slice
_PAIR_CLASSES = []       # unit -> weight-matrix class index
_CLS_LIST = []
for _u in range(_N_UNITS):
    _ta = _TILES[2 * _u]
    _ca = (_ta[1], _ta[3])
    if 2 * _u + 1 < len(_TILES):
        _cb = (_TILES[2 * _u + 1][1], _TILES[2 * _u + 1][3])
    else:
        _cb = None
    if (_ca, _cb) not in _CLS_LIST:
        _CLS_LIST.append((_ca, _cb))
    _PAIR_CLASSES.append(_CLS_LIST.index((_ca, _cb)))

# tile -> engine: even tiles on ScalarE (tanh), odd on VectorE (quintic);
# tile 17 flipped to ScalarE for load balance (ACT is faster per op).
_TILE_ON_ACT = {ti: (ti % 2 == 0) or ti == 17 for ti in range(len(_TILES))}


# ---------------- fits (input-independent constants) ----------------

def _taylor_sig(x):
    t = -x
    return 1.0 / (2.0 + t * (1.0 + t * (0.5 + t * ((1.0 / 6.0) + t * (1.0 / 24.0)))))


def _fit_consts():
    # conv quintic: c5 u'^5 + c3 u'^3 + c1 u' + beta ~= taylor_sig(2u),
    # u' = u + dq, u = 0.5*(v + b1)
    u = np.linspace(-2.05, 2.05, 2051)
    w = np.exp(-0.5 * (u / 0.36) ** 2) + 3e-4
    tgt = _taylor_sig(2 * u)
    sw = np.sqrt(w)
    best = None
    for dq in np.linspace(-0.3, 0.3, 61):
        uu = u + dq
        A = np.stack([uu ** 5, uu ** 3, uu, np.ones_like(u)], 1)
        coef, *_ = np.linalg.lstsq(A * sw[:, None], tgt * sw, rcond=None)
        e = A @ coef - tgt
        L = (w * e ** 2).sum()
        if best is None or L < best[0]:
            best = (L, dq, coef)
    _, dq, (c5, c3, c1, q_beta) = best

    # tanh: a*tanh(g t + d) + b ~= taylor_sig(t), t = v + b1
    t = np.linspace(-2.6, 2.6, 2601)
    wt = np.exp(-0.5 * (t / 0.70) ** 2) + 3e-4
    tt = _taylor_sig(t)
    swt = np.sqrt(wt)
    best = None
    for g in np.linspace(0.4, 0.75, 36):
        for dd in np.linspace(-0.3, 0.3, 31):
            A = np.stack([np.tanh(g * t + dd), np.ones_like(t)], 1)
            coef, *_ = np.linalg.lstsq(A * swt[:, None], tt * swt, rcond=None)
            e = A @ coef - tt
            L = (wt * e ** 2).sum()
            if best is None or L < best[0]:
                best = (L, g, dd, coef)
    _, tg, td, (t_alpha, t_beta) = best

    # h quintic (odd): a5 y^5 + a3 y^3 + a1 y ~= sigmoid(y) - 0.5
    y = np.linspace(-1.2, 1.2, 1201)
    wy = np.exp(-0.5 * (y / 0.30) ** 2) + 1e-3
    ty = 1.0 / (1.0 + np.exp(-y)) - 0.5
    swy = np.sqrt(wy)
    A = np.stack([y ** 5, y ** 3, y], 1)
    (a5, a3, a1), *_ = np.linalg.lstsq(A * swy[:, None], ty * swy, rcond=None)
    return dict(dq=float(dq), c5=float(c5), c3=float(c3), c1=float(c1),
                q_beta=float(q_beta), tg=float(tg), td=float(td),
                t_alpha=float(t_alpha), t_beta=float(t_beta),
                a5=float(a5), a3=float(a3), a1=float(a1))


_FC = _fit_consts()


# ---------------- host prep ----------------

def _kh(cls):
    return ((cls[0] + 2) * (cls[1] + 2) + 1) // 2


# quads of 4 tiles -> 4 concurrent 32-row-group DoubleRow conv matmuls
_N_QUADS = (len(_TILES) + 3) // 4
_QUAD_CLASSES = []
_QCLS_LIST = []
for _q in range(_N_QUADS):
    _key = tuple((_TILES[i][1], _TILES[i][3]) if i < len(_TILES) else None
                 for i in range(4 * _q, 4 * _q + 4))
    if _key not in _QCLS_LIST:
        _QCLS_LIST.append(_key)
    _QUAD_CLASSES.append(_QCLS_LIST.index(_key))


def _host_prep(x, w1, b1, w2, b2, fw1, fb1, fw2, fb2):
    x = np.asarray(x, np.float32)
    w1 = np.asarray(w1, np.float32); b1 = np.asarray(b1, np.float32)
    w2 = np.asarray(w2, np.float32); b2 = np.asarray(b2, np.float32)
    fw1 = np.asarray(fw1, np.float32); fb1 = np.asarray(fb1, np.float32)
    fw2 = np.asarray(fw2, np.float32); fb2 = np.asarray(fb2, np.float32)
    F = _FC

    def _banded(cls):
        noy, nox = cls
        ky, kx = noy + 2, nox + 2
        wt = np.zeros((ky * kx, 128), np.float32)
        for oy in range(noy):
            for ox in range(nox):
                for oc in range(6):
                    m = (oy * nox + ox) * 6 + oc
                    for dy in range(3):
                        for dx in range(3):
                            wt[(oy + dy) * kx + (ox + dx), m] = \
                                _A_IN * w1[oc, 0, dy, dx]
        return wt

    # quad-class DoubleRow conv weights: tile i of a quad at partitions
    # 32i:32i+Kh, cols qc*256 + j*128 + f holds wt[j*Kh + p, f]
    w1pack = np.zeros((128, len(_QCLS_LIST) * 256), np.float32)
    for qc, key in enumerate(_QCLS_LIST):
        for i, cls in enumerate(key):
            if cls is None:
                continue
            wt = _banded(cls)
            K = wt.shape[0]
            kh = _kh(cls)
            wsplit = np.zeros((2 * kh, 128), np.float32)
            wsplit[:K] = wt
            for j in range(2):
                w1pack[32 * i:32 * i + kh,
                       qc * 256 + j * 128: qc * 256 + (j + 1) * 128] = \
                    wsplit[j * kh:(j + 1) * kh]

    # fold conv2 + fc1 -> Wc [128, 6*26*26], bias bcomb
    fw1r = fw1.reshape(128, 7, 24, 24)
    Wc = np.zeros((128, 6, 26, 26), np.float32)
    for dy in range(3):
        for dx in range(3):
            Wc[:, :, dy:dy + 24, dx:dx + 24] += np.einsum(
                "joyx,oi->jiyx", fw1r, w2[:, :, dy, dx], optimize=True)
    bcomb = fb1 + np.einsum("joyx,o->j", fw1r, b2)
    Wc_flat = Wc.reshape(128, 6 * 26 * 26)

    # wcpack [128, 18*256] fp8 (DoubleRow layout: unit u, j in {0,1} at
    # cols u*256 + j*128 + f), per-tile gain G*alpha; beta folds into bias
    wcpack = np.zeros((128, _N_UNITS * 256), np.float32)
    bc_eff = bcomb.copy()
    for ti, t in enumerate(_TILES):
        oy0, noy, ox0, nox = t
        M = noy * nox * 6
        unit, j = ti // 2, ti % 2
        alpha = F["t_alpha"] if _TILE_ON_ACT[ti] else 1.0
        beta = F["t_beta"] if _TILE_ON_ACT[ti] else F["q_beta"]
        cols = []
        for oy in range(noy):
            for ox in range(nox):
                for oc in range(6):
                    cols.append((oc * 26 + oy0 + oy) * 26 + ox0 + ox)
        Wt = Wc_flat[:, cols]                       # [128 feat, M]
        wcpack[0:M, unit * 256 + j * 128: unit * 256 + (j + 1) * 128] = \
            (_G * alpha) * Wt.T
        bc_eff += beta * Wt.sum(axis=1)

    bias_act = (F["tg"] * b1[np.arange(128) % 6] + F["td"]).astype(np.float32)
    bias_dve = (_A_IN * b1[np.arange(128) % 6] + F["dq"]).astype(np.float32)
    cvec = np.stack([np.full(128, F["c5"], np.float32),
                     np.full(128, F["a5"] / _G ** 5, np.float32)], 1)
    fb2e = fb2 + 0.5 * fw2.sum(axis=1)

    consts = dict(
        w1pack=w1pack.astype(_F8), wcpack=wcpack.astype(_F8),
        biasact=bias_act.reshape(128, 1),
        biasdve=bias_dve.reshape(128, 1),
        cvec=cvec,
        bch=(_G * bc_eff).reshape(128, 1).astype(np.float32),
        fw2t=np.ascontiguousarray(fw2.T).astype(np.float16),
        fb2r=np.tile(fb2e.reshape(1, 10), (128, 4)).astype(np.float32),
    )

    # pre-windowed fp8 input slab per core: [128, nsl*nquads*1024].
    # Quad block (sl, q) at cols (sl*nq+q)*1024; tile i of the quad at
    # partitions 32i:32i+Kh with its split window rows as two 512-col
    # j-blocks (DoubleRow layout).
    x_pm = x.reshape(_B, 784).T.astype(_F8)         # [784, B]
    x_pm_pad = np.zeros((785, _B), _F8)             # row 784 stays zero pad
    x_pm_pad[:784] = x_pm
    slabs = []
    for c in range(_NCORES):
        slab = np.zeros((128, _NSL * _N_QUADS * 2 * _SLICE), _F8)
        for sl in range(_NSL):
            s0 = c * _PC + sl * _SLICE
            for ti, t in enumerate(_TILES):
                oy0, noy, ox0, nox = t
                ky, kx = noy + 2, nox + 2
                q, i = ti // 4, ti % 4
                kh = _kh((noy, nox))
                rows = ((np.arange(ky)[:, None] + oy0) * 28 +
                        (np.arange(kx)[None, :] + ox0)).reshape(-1)
                rows = np.concatenate(
                    [rows, np.full(2 * kh - ky * kx, 784, np.int64)])
                cb = (sl * _N_QUADS + q) * 2 * _SLICE
                for j in range(2):
                    slab[32 * i:32 * i + kh,
                         cb + j * _SLICE: cb + (j + 1) * _SLICE] = \
                        x_pm_pad[rows[j * kh:(j + 1) * kh], s0:s0 + _SLICE]
        slabs.append(slab)
    return slabs, consts


# ---------------- custom DVE op: odd quintic ----------------

def _register_sigpoly():
    import concourse.dve_ops as dve_ops
    if "SIGPOLY5_ANT" in dve_ops._SUB_OPCODE_FOR_NAME:
        return next(o for o in dve_ops.OPS if o.name == "SIGPOLY5_ANT")
    from concourse.dve_spec import (Spec, Src0, C0, C1, C2, C3, lower,
                                    _spill_c3_to_src1)
    from concourse.dve_uop import DveOpSpec

    # u = in0 + s0;  out = ((c5*u^2 + s1)*u^2 + imm2)*u   (c5 via in1)
    u = Src0 + C0
    w = u * u
    body = _spill_c3_to_src1(((C3 * w + C1) * w + C2) * u)

    def _ref(in0, in1, s0, s1, imm2):
        uu = in0.astype(np.float32) + s0
        ww = uu * uu
        c5 = np.asarray(in1, np.float32).reshape(in0.shape[0], -1)[:, :1]
        return ((c5 * ww + s1) * ww + imm2) * uu

    spec = Spec(body=body, reference=_ref)
    name = "SIGPOLY5_ANT"
    row = max(dve_ops._SUB_OPCODE_FOR_NAME.values()) + 1
    assert row < 0x20
    dve_ops._SUB_OPCODE_FOR_NAME[name] = row
    shas = {}
    for ver in ("v3", "v4"):
        tmp = DveOpSpec(name=name, opcode=row, uops=lower(spec, ver=ver),
                        rd1_en=True)
        shas[ver] = tmp.sha(ver)
    op = dve_ops.DveOp(name, spec, subdim=False, uops_sha=shas)
    dve_ops.OPS.append(op)
    dve_ops.CUSTOM_DVE_SPECS[name] = spec
    return op


def _pin_act_tables():
    """Pin Tanh -> exp_and_others, Exp/Ln -> natural_log_exp_and_others
    so the kernel costs exactly two ACT table loads."""
    import concourse.bacc as bacc
    import concourse.mybir as mybir
    if getattr(bacc, "_ant_tables_pinned", False):
        return
    orig = bacc.get_activation_tables
    AF = mybir.ActivationFunctionType

    def patched(arch):
        tabs = {k: set(v) for k, v in orig(arch).items()}
        for name, fns in tabs.items():
            if name != "exp_and_others":
                fns.discard(AF.Tanh)
            if name != "natural_log_exp_and_others":
                fns.discard(AF.Exp)
                fns.discard(AF.Ln)
        return tabs

    bacc.get_activation_tables = patched
    bacc._ant_tables_pinned = True


# ---------------- program ----------------

def _build_program():
    import concourse.bacc as bacc
    import concourse.mybir as mybir
    from concourse.tile import TileContext
    from concourse.tile_rust import add_dep_helper
    from concourse.alu_op_type import AluOpType

    f32 = mybir.dt.float32
    f16 = mybir.dt.float16
    f8 = mybir.dt.float8e4
    AF = mybir.ActivationFunctionType
    DR = mybir.MatmulPerfMode.DoubleRow
    sigpoly = _register_sigpoly()
    _pin_act_tables()
    F = _FC

    nc = bacc.Bacc()
    n_cols = _NSL * _N_QUADS * 2 * _SLICE
    xwin_d = nc.declare_dram_parameter("xwin", [128, n_cols], f8, isOutput=False)
    w1pack_d = nc.declare_dram_parameter("w1pack", [128, len(_QCLS_LIST) * 256],
                                         f8, isOutput=False)
    wcpack_d = nc.declare_dram_parameter("wcpack", [128, _N_UNITS * 256], f8,
                                         isOutput=False)
    biasact_d = nc.declare_dram_parameter("biasact", [128, 1], f32, isOutput=False)
    biasdve_d = nc.declare_dram_parameter("biasdve", [128, 1], f32, isOutput=False)
    cvec_d = nc.declare_dram_parameter("cvec", [128, 2], f32, isOutput=False)
    bch_d = nc.declare_dram_parameter("bch", [128, 1], f32, isOutput=False)
    fw2t_d = nc.declare_dram_parameter("fw2t", [128, 10], f16, isOutput=False)
    fb2r_d = nc.declare_dram_parameter("fb2r", [128, 40], f32, isOutput=False)
    out_d = nc.declare_dram_parameter("out", [_PC, 10], f32, isOutput=True)

    n_chunks = 6
    ch_cols = n_cols // n_chunks        # 3072 = 6 units

    with TileContext(nc) as tc:
        with (
            tc.tile_pool(name="const", bufs=1) as cpool,
            tc.tile_pool(name="work", bufs=3) as wpool,
            tc.tile_pool(name="cps", bufs=6, space="PSUM") as cps,
            tc.tile_pool(name="zps", bufs=1, space="PSUM") as zps,
            tc.tile_pool(name="fps", bufs=1, space="PSUM") as fps,
        ):
            # weights/consts on the scalar HWDGE ring (w1pack/wcpack first)
            w1pack_sb = cpool.tile([128, len(_QCLS_LIST) * 256], f8,
                                   tag="w1p", name="w1pack_sb", bufs=1)
            nc.scalar.dma_start(out=w1pack_sb, in_=w1pack_d[:])
            wcpack_sb = cpool.tile([128, _N_UNITS * 256], f8, tag="wcp",
                                   name="wcpack_sb", bufs=1)
            nc.scalar.dma_start(out=wcpack_sb, in_=wcpack_d[:])
            small = []
            for nm, par, shape, dt_ in (
                ("biasact", biasact_d, [128, 1], f32),
                ("biasdve", biasdve_d, [128, 1], f32),
                ("cvec", cvec_d, [128, 2], f32),
                ("bch", bch_d, [128, 1], f32),
                ("fw2t", fw2t_d, [128, 10], f16),
                ("fb2r", fb2r_d, [128, 40], f32),
            ):
                t = cpool.tile(shape, dt_, tag=nm, name=nm + "_sb", bufs=1)
                nc.scalar.dma_start(out=t, in_=par[:])
                small.append(t)
            biasact_sb, biasdve_sb, cvec_sb, bch_sb, fw2t_sb, fb2r_sb = small

            # xwin slab chunks on the sync ring
            xw = []
            for j in range(n_chunks):
                t = cpool.tile([128, ch_cols], f8, tag="xw", name=f"xw{j}",
                               bufs=n_chunks)
                nc.sync.dma_start(out=t, in_=xwin_d[:, j * ch_cols:(j + 1) * ch_cols])
                xw.append(t)

            tanh_insts = []
            zs = []
            pe_chain = [None]

            def _pe(inst):
                if pe_chain[0] is not None:
                    add_dep_helper(inst.ins, pe_chain[0].ins, sync=False,
                                   reason="pe order")
                pe_chain[0] = inst
                return inst

            for sl in range(_NSL):
                z = zps.tile([128, _SLICE], f32, tag="z", name=f"z{sl}")
                zs.append(z)
                s_tiles = {}
                pend_z = []

                def _emit_z(u, z=z):
                    s = s_tiles.pop(u)
                    single = (2 * u + 1 >= len(_TILES))
                    if single:
                        _pe(nc.tensor.matmul(
                            z, wcpack_sb[:, u * 256:u * 256 + 128],
                            s[:, 0:_SLICE], start=(u == 0),
                            stop=(u == _N_UNITS - 1)))
                    else:
                        _pe(nc.tensor.matmul(
                            z,
                            wcpack_sb[:, u * 256:(u + 1) * 256].rearrange(
                                "p (j f) -> p j f", j=2),
                            s.rearrange("p (j n) -> p j n", j=2),
                            start=(u == 0), stop=(u == _N_UNITS - 1),
                            perf_mode=DR))

                for q in range(_N_QUADS):
                    qc = _QUAD_CLASSES[q]
                    cb = (sl * _N_QUADS + q) * 2 * _SLICE
                    ch, lo = cb // ch_cols, cb % ch_cols
                    ntiles = min(4, len(_TILES) - 4 * q)
                    # conv wall: up to 4 concurrent DoubleRow MMs
                    cpt = []
                    for i in range(ntiles):
                        t = _TILES[4 * q + i]
                        kh = _kh((t[1], t[3]))
                        cp = cps.tile([128, _SLICE], f32, tag="cp",
                                      name=f"cp{sl}_{4 * q + i}")
                        cpt.append(cp)
                        _pe(nc.tensor.matmul(
                            cp,
                            w1pack_sb[32 * i:32 * i + kh,
                                      qc * 256:(qc + 1) * 256].rearrange(
                                          "p (j f) -> p j f", j=2),
                            xw[ch][32 * i:32 * i + kh,
                                   lo:lo + 2 * _SLICE].rearrange(
                                       "p (j n) -> p j n", j=2),
                            start=True, stop=True, perf_mode=DR,
                            tile_position=(32 * i, 0)))
                    # nonlinearity per tile, pair-shaped s output
                    for h in range((ntiles + 1) // 2):
                        u = 2 * q + h
                        s = wpool.tile([128, 2 * _SLICE], f8, tag="s",
                                       name=f"s{sl}_{u}")
                        s_tiles[u] = s
                        for jj in range(min(2, ntiles - 2 * h)):
                            ti = 4 * q + 2 * h + jj
                            cp = cpt[2 * h + jj]
                            dst = s[:, jj * _SLICE:(jj + 1) * _SLICE]
                            if _TILE_ON_ACT[ti]:
                                ti_ = nc.scalar.activation(
                                    dst, cp, AF.Tanh,
                                    bias=biasact_sb[:], scale=F["tg"] / _A_IN)
                                tanh_insts.append(ti_)
                            else:
                                nc.vector._custom_dve(
                                    sigpoly, out=dst, in0=cp,
                                    in1=cvec_sb[:, 0:1], s0=biasdve_sb[:],
                                    s1=F["c3"], imm2=F["c1"])
                        pend_z.append(u)
                    # delayed z block: previous quad's two units back-to-back
                    while len(pend_z) > 2:
                        _emit_z(pend_z.pop(0))
                while pend_z:
                    _emit_z(pend_z.pop(0))

            # ---- tail: h = (sigma-0.5) via quintic, fc2, log_softmax ----
            last_tanh = tanh_insts[-1]
            for sl in range(_NSL):
                hp = wpool.tile([128, _SLICE], f16, tag="h", name=f"h{sl}")
                nc.vector._custom_dve(
                    sigpoly, out=hp, in0=zs[sl], in1=cvec_sb[:, 1:2],
                    s0=bch_sb[:], s1=F["a3"] / _G ** 3, imm2=F["a1"] / _G)
                ng = _SLICE // 128
                fp = fps.tile([128, 10 * ng], f32, tag="fp", name=f"fp{sl}",
                              bufs=1)
                for g in range(ng):
                    nc.tensor.matmul(fp[:, g * 10:(g + 1) * 10],
                                     hp[:, g * 128:(g + 1) * 128],
                                     fw2t_sb[:], start=True, stop=True)
                lg = wpool.tile([128, 10 * ng], f32, tag="lg", name=f"lg{sl}")
                nc.vector.tensor_tensor(out=lg, in0=fp, in1=fb2r_sb[:, 0:10 * ng],
                                        op=AluOpType.add)
                e = wpool.tile([128, 10 * ng], f32, tag="e", name=f"e{sl}")
                ei = nc.scalar.activation(e, lg, AF.Exp)
                add_dep_helper(ei.ins, last_tanh.ins, sync=False,
                               reason="exp after last tanh (table sets)")
                ssum = wpool.tile([128, ng], f32, tag="ss", name=f"ss{sl}")
                nc.vector.tensor_reduce(
                    ssum, e.rearrange("p (g k) -> p g k", k=10),
                    axis=mybir.AxisListType.X, op=AluOpType.add)
                lns = wpool.tile([128, ng], f32, tag="ls", name=f"ls{sl}")
                li = nc.scalar.activation(lns, ssum, AF.Ln)
                add_dep_helper(li.ins, last_tanh.ins, sync=False,
                               reason="ln after last tanh (table sets)")
                ot = wpool.tile([128, 10 * ng], f32, tag="ot", name=f"ot{sl}")
                for g in range(ng):
                    nc.vector.tensor_scalar(
                        out=ot[:, g * 10:(g + 1) * 10],
                        in0=lg[:, g * 10:(g + 1) * 10],
                        scalar1=lns[:, g:g + 1], scalar2=None,
                        op0=AluOpType.subtract)
                orow = sl * _SLICE
                nc.sync.dma_start(
                    out=out_d[orow:orow + _SLICE, :].rearrange(
                        "(g p) k -> p g k", p=128),
                    in_=ot.rearrange("p (g k) -> p g k", k=10))
    nc.compile()
    return nc


_PROGRAM_CACHE = {}


def kernel(x, w1, b1, w2, b2, fw1, fb1, fw2, fb2):
    global LAST_RESULTS
    slabs, consts = _host_prep(x, w1, b1, w2, b2, fw1, fb1, fw2, fb2)

    if "nc" not in _PROGRAM_CACHE:
        _PROGRAM_CACHE["nc"] = _build_program()
    nc = _PROGRAM_CACHE["nc"]

    in_maps = []
    for c in range(_NCORES):
        m = dict(consts)
        m["xwin"] = slabs[c]
        in_maps.append(m)

    from concourse.bass_utils import run_bass_kernel_spmd
    trace = bool(int(os.environ.get("BASS_KERNEL_TRACE", "0")))
    res = run_bass_kernel_spmd(nc, in_maps, core_ids=list(range(_NCORES)),
                               trace=trace)
    LAST_RESULTS = res
    return np.concatenate([r["out"] for r in res.results], axis=0)


# revision 19
# speedup vs baseline: 1.8294x; 1.0849x over previous
"""Trainium2 Bass kernel for nn_Net_39230231281866 (dense_cnn), v2.

Network: conv3x3(1->6) -> Taylor-sigmoid -> conv3x3(6->7) -> flatten
         -> fc(4032->128) -> sigmoid -> fc(128->10) -> log_softmax,
batch 8192, data-parallel over 8 NeuronCores (1024 samples/core).

v2 design (vs v1 baseline ~99us):
  * conv1: fp8 banded matmuls, TWO tiles packed concurrently in the PE
    array via row-groups (K<=48 -> tiles at base_partition 0 and 64,
    tile_position auto-derived). 35-tile cover of the 26x26 output.
  * Taylor-sigmoid replaced by ONE pass per psum pair, statically split
    between ScalarE (tanh, affine-fitted to the Taylor sigmoid) and
    VectorE (custom single-pass odd-quintic DVE op fitted likewise).
    Outputs are fp8e4 s-tiles.
  * conv2+fc1 folded into W_comb as before, but quantized fp8e4 with a
    global gain G folded out through the fc1-sigmoid, and the 36 K-chunks
    contracted as 18 DoubleRow fp8 matmuls (K=256 per instruction).
  * fc1 sigmoid computed as (sigma-0.5) via the same quintic DVE op, so
    ScalarE's exp/ln table load can hide under the end of the main loop;
    0.5 is folded into fb2. fc2 in fp16.
  * DMA: one fp8 pre-windowed input slab per core, loaded in 6 large
    chunk DMAs on the sync ring; weights on the scalar ring.
"""

import os
import numpy as np
import ml_dtypes

_B = 8192
_NCORES = 8
_PC = _B // _NCORES
_SLICE = 512
_NSL = _PC // _SLICE

_A_IN = 0.5          # conv matmul gain: cp = _A_IN * conv1(x)
_G = 64.0            # global fp8 gain for W_comb
_F8 = ml_dtypes.float8_e4m3

LAST_RESULTS = None


# ---------------- tiling ----------------

def _tiles():
    """35 rectangular tiles covering the 26x26 conv1 output, all with
    M = 6*noy*nox <= 128 and K = (noy+2)*(nox+2) <= 48 (2 fit in the PE
    rows as 64-row groups). Sorted by class so pairs share a class."""
    ts = []
    for oy0 in range(0, 24, 3):
        for (ox0, nox) in [(0, 7), (7, 7), (14, 7), (21, 5)]:
            ts.append((oy0, 3, ox0, nox))
    for (ox0, nox) in [(0, 10), (10, 10), (20, 6)]:
        ts.append((24, 2, ox0, nox))
    order = {(3, 7): 0, (3, 5): 1, (2, 10): 2, (2, 6): 3}
    ts.sort(key=lambda t: order[(t[1], t[3])])
    return ts


_TILES = _tiles()
_N_UNITS = 18            # 17 pairs + 1 single, per slice
_PAIR_CLASSES = []       # unit -> weight-matrix class index
_CLS_LIST = []
for _u in range(_N_UNITS):
    _ta = _TILES[2 * _u]
    _ca = (_ta[1], _ta[3])
    if 2 * _u + 1 < len(_TILES):
        _cb = (_TILES[2 * _u + 1][1], _TILES[2 * _u + 1][3])
    else:
        _cb = None
    if (_ca, _cb) not in _CLS_LIST:
        _CLS_LIST.append((_ca, _cb))
    _PAIR_CLASSES.append(_CLS_LIST.index((_ca, _cb)))

# tile -> engine: even tiles on ScalarE (tanh), odd on VectorE (quintic);
# tile 17 flipped to ScalarE for load balance (ACT is faster per op).
_TILE_ON_ACT = {ti: (ti % 2 == 0) or ti == 17 for ti in range(len(_TILES))}


# ---------------- fits (input-independent constants) ----------------

def _taylor_sig(x):
    t = -x
    return 1.0 / (2.0 + t * (1.0 + t * (0.5 + t * ((1.0 / 6.0) + t * (1.0 / 24.0)))))


def _fit_consts():
    # conv quintic: c5 u'^5 + c3 u'^3 + c1 u' + beta ~= taylor_sig(2u),
    # u' = u + dq, u = 0.5*(v + b1)
    u = np.linspace(-2.05, 2.05, 2051)
    w = np.exp(-0.5 * (u / 0.36) ** 2) + 3e-4
    tgt = _taylor_sig(2 * u)
    sw = np.sqrt(w)
    best = None
    for dq in np.linspace(-0.3, 0.3, 61):
        uu = u + dq
        A = np.stack([uu ** 5, uu ** 3, uu, np.ones_like(u)], 1)
        coef, *_ = np.linalg.lstsq(A * sw[:, None], tgt * sw, rcond=None)
        e = A @ coef - tgt
        L = (w * e ** 2).sum()
        if best is None or L < best[0]:
            best = (L, dq, coef)
    _, dq, (c5, c3, c1, q_beta) = best

    # tanh: a*tanh(g t + d) + b ~= taylor_sig(t), t = v + b1
    t = np.linspace(-2.6, 2.6, 2601)
    wt = np.exp(-0.5 * (t / 0.70) ** 2) + 3e-4
    tt = _taylor_sig(t)
    swt = np.sqrt(wt)
    best = None
    for g in np.linspace(0.4, 0.75, 36):
        for dd in np.linspace(-0.3, 0.3, 31):
            A = np.stack([np.tanh(g * t + dd), np.ones_like(t)], 1)
            coef, *_ = np.linalg.lstsq(A * swt[:, None], tt * swt, rcond=None)
            e = A @ coef - tt
            L = (wt * e ** 2).sum()
            if best is None or L < best[0]:
                best = (L, g, dd, coef)
    _, tg, td, (t_alpha, t_beta) = best

    # h quintic (odd): a5 y^5 + a3 y^3 + a1 y ~= sigmoid(y) - 0.5
    y = np.linspace(-1.2, 1.2, 1201)
    wy = np.exp(-0.5 * (y / 0.30) ** 2) + 1e-3
    ty = 1.0 / (1.0 + np.exp(-y)) - 0.5
    swy = np.sqrt(wy)
    A = np.stack([y ** 5, y ** 3, y], 1)
    (a5, a3, a1), *_ = np.linalg.lstsq(A * swy[:, None], ty * swy, rcond=None)
    return dict(dq=float(dq), c5=float(c5), c3=float(c3), c1=float(c1),
                q_beta=float(q_beta), tg=float(tg), td=float(td),
                t_alpha=float(t_alpha), t_beta=float(t_beta),
                a5=float(a5), a3=float(a3), a1=float(a1))


_FC = _fit_consts()


# ---------------- host prep ----------------

def _kh(cls):
    return ((cls[0] + 2) * (cls[1] + 2) + 1) // 2


# quads of 4 tiles -> 4 concurrent 32-row-group DoubleRow conv matmuls
_N_QUADS = (len(_TILES) + 3) // 4
_QUAD_CLASSES = []
_QCLS_LIST = []
for _q in range(_N_QUADS):
    _key = tuple((_TILES[i][1], _TILES[i][3]) if i < len(_TILES) else None
                 for i in range(4 * _q, 4 * _q + 4))
    if _key not in _QCLS_LIST:
        _QCLS_LIST.append(_key)
    _QUAD_CLASSES.append(_QCLS_LIST.index(_key))


def _host_prep(x, w1, b1, w2, b2, fw1, fb1, fw2, fb2):
    x = np.asarray(x, np.float32)
    w1 = np.asarray(w1, np.float32); b1 = np.asarray(b1, np.float32)
    w2 = np.asarray(w2, np.float32); b2 = np.asarray(b2, np.float32)
    fw1 = np.asarray(fw1, np.float32); fb1 = np.asarray(fb1, np.float32)
    fw2 = np.asarray(fw2, np.float32); fb2 = np.asarray(fb2, np.float32)
    F = _FC

    def _banded(cls):
        noy, nox = cls
        ky, kx = noy + 2, nox + 2
        wt = np.zeros((ky * kx, 128), np.float32)
        for oy in range(noy):
            for ox in range(nox):
                for oc in range(6):
                    m = (oy * nox + ox) * 6 + oc
                    for dy in range(3):
                        for dx in range(3):
                            wt[(oy + dy) * kx + (ox + dx), m] = \
                                _A_IN * w1[oc, 0, dy, dx]
        return wt

    # quad-class DoubleRow conv weights: tile i of a quad at partitions
    # 32i:32i+Kh, cols qc*256 + j*128 + f holds wt[j*Kh + p, f]
    w1pack = np.zeros((128, len(_QCLS_LIST) * 256), np.float32)
    for qc, key in enumerate(_QCLS_LIST):
        for i, cls in enumerate(key):
            if cls is None:
                continue
            wt = _banded(cls)
            K = wt.shape[0]
            kh = _kh(cls)
            wsplit = np.zeros((2 * kh, 128), np.float32)
            wsplit[:K] = wt
            for j in range(2):
                w1pack[32 * i:32 * i + kh,
                       qc * 256 + j * 128: qc * 256 + (j + 1) * 128] = \
                    wsplit[j * kh:(j + 1) * kh]

    # fold conv2 + fc1 -> Wc [128, 6*26*26], bias bcomb
    fw1r = fw1.reshape(128, 7, 24, 24)
    Wc = np.zeros((128, 6, 26, 26), np.float32)
    for dy in range(3):
        for dx in range(3):
            Wc[:, :, dy:dy + 24, dx:dx + 24] += np.einsum(
                "joyx,oi->jiyx", fw1r, w2[:, :, dy, dx], optimize=True)
    bcomb = fb1 + np.einsum("joyx,o->j", fw1r, b2)
    Wc_flat = Wc.reshape(128, 6 * 26 * 26)

    # wcpack [128, 18*256] fp8 (DoubleRow layout: unit u, j in {0,1} at
    # cols u*256 + j*128 + f), per-tile gain G*alpha; beta folds into bias
    wcpack = np.zeros((128, _N_UNITS * 256), np.float32)
    bc_eff = bcomb.copy()
    for ti, t in enumerate(_TILES):
        oy0, noy, ox0, nox = t
        M = noy * nox * 6
        unit, j = ti // 2, ti % 2
        alpha = F["t_alpha"] if _TILE_ON_ACT[ti] else 1.0
        beta = F["t_beta"] if _TILE_ON_ACT[ti] else F["q_beta"]
        cols = []
        for oy in range(noy):
            for ox in range(nox):
                for oc in range(6):
                    cols.append((oc * 26 + oy0 + oy) * 26 + ox0 + ox)
        Wt = Wc_flat[:, cols]                       # [128 feat, M]
        wcpack[0:M, unit * 256 + j * 128: unit * 256 + (j + 1) * 128] = \
            (_G * alpha) * Wt.T
        bc_eff += beta * Wt.sum(axis=1)

    bias_act = (F["tg"] * b1[np.arange(128) % 6] + F["td"]).astype(np.float32)
    bias_dve = (_A_IN * b1[np.arange(128) % 6] + F["dq"]).astype(np.float32)
    fb2e = fb2 + 0.5 * fw2.sum(axis=1)

    # all f32 per-partition consts packed into one [128, 45] DMA:
    # col 0 biasact, 1 biasdve, 2 c5, 3 a5/G^5, 4 G*bc_eff, 5:45 fb2r
    cpack = np.zeros((128, 45), np.float32)
    cpack[:, 0] = bias_act
    cpack[:, 1] = bias_dve
    cpack[:, 2] = F["c5"]
    cpack[:, 3] = F["a5"] / _G ** 5
    cpack[:, 4] = _G * bc_eff
    cpack[:, 5:45] = np.tile(fb2e.reshape(1, 10), (128, 4))

    consts = dict(
        w1pack=w1pack.astype(_F8), wcpack=wcpack.astype(_F8),
        cpack=cpack,
        fw2t=np.ascontiguousarray(fw2.T).astype(np.float16),
    )

    # pre-windowed fp8 input slab per core: [128, nsl*nquads*1024].
    # Quad block (sl, q) at cols (sl*nq+q)*1024; tile i of the quad at
    # partitions 32i:32i+Kh with its split window rows as two 512-col
    # j-blocks (DoubleRow layout).
    x_pm = x.reshape(_B, 784).T.astype(_F8)         # [784, B]
    x_pm_pad = np.zeros((785, _B), _F8)             # row 784 stays zero pad
    x_pm_pad[:784] = x_pm
    slabs = []
    for c in range(_NCORES):
        slab = np.zeros((128, _NSL * _N_QUADS * 2 * _SLICE), _F8)
        for sl in range(_NSL):
            s0 = c * _PC + sl * _SLICE
            for ti, t in enumerate(_TILES):
                oy0, noy, ox0, nox = t
                ky, kx = noy + 2, nox + 2
                q, i = ti // 4, ti % 4
                kh = _kh((noy, nox))
                rows = ((np.arange(ky)[:, None] + oy0) * 28 +
                        (np.arange(kx)[None, :] + ox0)).reshape(-1)
                rows = np.concatenate(
                    [rows, np.full(2 * kh - ky * kx, 784, np.int64)])
                cb = (sl * _N_QUADS + q) * 2 * _SLICE
                for j in range(2):
                    slab[32 * i:32 * i + kh,
                         cb + j * _SLICE: cb + (j + 1) * _SLICE] = \
                        x_pm_pad[rows[j * kh:(j + 1) * kh], s0:s0 + _SLICE]
        slabs.append(slab)
    return slabs, consts


# ---------------- custom DVE op: odd quintic ----------------

def _register_sigpoly():
    import concourse.dve_ops as dve_ops
    if "SIGPOLY5_ANT" in dve_ops._SUB_OPCODE_FOR_NAME:
        return next(o for o in dve_ops.OPS if o.name == "SIGPOLY5_ANT")
    from concourse.dve_spec import (Spec, Src0, C0, C1, C2, C3, lower,
                                    _spill_c3_to_src1)
    from concourse.dve_uop import DveOpSpec

    # u = in0 + s0;  out = ((c5*u^2 + s1)*u^2 + imm2)*u   (c5 via in1)
    u = Src0 + C0
    w = u * u
    body = _spill_c3_to_src1(((C3 * w + C1) * w + C2) * u)

    def _ref(in0, in1, s0, s1, imm2):
        uu = in0.astype(np.float32) + s0
        ww = uu * uu
        c5 = np.asarray(in1, np.float32).reshape(in0.shape[0], -1)[:, :1]
        return ((c5 * ww + s1) * ww + imm2) * uu

    spec = Spec(body=body, reference=_ref)
    name = "SIGPOLY5_ANT"
    row = max(dve_ops._SUB_OPCODE_FOR_NAME.values()) + 1
    assert row < 0x20
    dve_ops._SUB_OPCODE_FOR_NAME[name] = row
    shas = {}
    for ver in ("v3", "v4"):
        tmp = DveOpSpec(name=name, opcode=row, uops=lower(spec, ver=ver),
                        rd1_en=True)
        shas[ver] = tmp.sha(ver)
    op = dve_ops.DveOp(name, spec, subdim=False, uops_sha=shas)
    dve_ops.OPS.append(op)
    dve_ops.CUSTOM_DVE_SPECS[name] = spec
    return op


def _pin_act_tables():
    """Pin Tanh -> exp_and_others, Exp/Ln -> natural_log_exp_and_others
    so the kernel costs exactly two ACT table loads."""
    import concourse.bacc as bacc
    import concourse.mybir as mybir
    if getattr(bacc, "_ant_tables_pinned", False):
        return
    orig = bacc.get_activation_tables
    AF = mybir.ActivationFunctionType

    def patched(arch):
        tabs = {k: set(v) for k, v in orig(arch).items()}
        for name, fns in tabs.items():
            if name != "exp_and_others":
                fns.discard(AF.Tanh)
            if name != "natural_log_exp_and_others":
                fns.discard(AF.Exp)
                fns.discard(AF.Ln)
        return tabs

    bacc.get_activation_tables = patched
    bacc._ant_tables_pinned = True


# ---------------- program ----------------

def _build_program():
    import concourse.bacc as bacc
    import concourse.mybir as mybir
    from concourse.tile import TileContext
    from concourse.tile_rust import add_dep_helper
    from concourse.alu_op_type import AluOpType

    f32 = mybir.dt.float32
    f16 = mybir.dt.float16
    f8 = mybir.dt.float8e4
    AF = mybir.ActivationFunctionType
    DR = mybir.MatmulPerfMode.DoubleRow
    sigpoly = _register_sigpoly()
    _pin_act_tables()
    F = _FC

    nc = bacc.Bacc()
    n_cols = _NSL * _N_QUADS * 2 * _SLICE
    xwin_d = nc.declare_dram_parameter("xwin", [128, n_cols], f8, isOutput=False)
    w1pack_d = nc.declare_dram_parameter("w1pack", [128, len(_QCLS_LIST) * 256],
                                         f8, isOutput=False)
    wcpack_d = nc.declare_dram_parameter("wcpack", [128, _N_UNITS * 256], f8,
                                         isOutput=False)
    cpack_d = nc.declare_dram_parameter("cpack", [128, 45], f32, isOutput=False)
    fw2t_d = nc.declare_dram_parameter("fw2t", [128, 10], f16, isOutput=False)
    out_d = nc.declare_dram_parameter("out", [_PC, 10], f32, isOutput=True)

    # xwin chunk boundaries in cols (multiples of 1024 = one quad block);
    # small leading chunks so the first conv starts early
    ch_bounds = [0, 1024, 3072, 6144, 10240, 14336, n_cols]

    with TileContext(nc) as tc:
        with (
            tc.tile_pool(name="const", bufs=1) as cpool,
            tc.tile_pool(name="work", bufs=3) as wpool,
            tc.tile_pool(name="cps", bufs=6, space="PSUM") as cps,
            tc.tile_pool(name="zps", bufs=1, space="PSUM") as zps,
            tc.tile_pool(name="fps", bufs=1, space="PSUM") as fps,
        ):
            # weights on the scalar HWDGE ring, consts+input on the sync ring
            w1pack_sb = cpool.tile([128, len(_QCLS_LIST) * 256], f8,
                                   tag="w1p", name="w1pack_sb", bufs=1)
            nc.scalar.dma_start(out=w1pack_sb, in_=w1pack_d[:])
            wcpack_sb = cpool.tile([128, _N_UNITS * 256], f8, tag="wcp",
                                   name="wcpack_sb", bufs=1)
            nc.scalar.dma_start(out=wcpack_sb, in_=wcpack_d[:])
            cpack_sb = cpool.tile([128, 45], f32, tag="cpk", name="cpack_sb",
                                  bufs=1)
            nc.sync.dma_start(out=cpack_sb, in_=cpack_d[:])
            fw2t_sb = cpool.tile([128, 10], f16, tag="fw2t", name="fw2t_sb",
                                 bufs=1)
            nc.sync.dma_start(out=fw2t_sb, in_=fw2t_d[:])
            biasact_sb = cpack_sb[:, 0:1]
            biasdve_sb = cpack_sb[:, 1:2]
            cvec_sb = cpack_sb[:, 2:4]
            bch_sb = cpack_sb[:, 4:5]
            fb2r_sb = cpack_sb[:, 5:45]

            # xwin slab chunks on the sync ring
            xw = []          # (tile, col0, ncols) per chunk
            for j in range(len(ch_bounds) - 1):
                c0, c1 = ch_bounds[j], ch_bounds[j + 1]
                t = cpool.tile([128, c1 - c0], f8, tag=f"xw{j}",
                               name=f"xw{j}", bufs=1)
                nc.sync.dma_start(out=t, in_=xwin_d[:, c0:c1])
                xw.append((t, c0, c1 - c0))

            def _xw_ap(cb, width):
                for t, c0, nc_ in xw:
                    if c0 <= cb and cb + width <= c0 + nc_:
                        return t, cb - c0
                raise AssertionError("chunk boundary crosses quad block")

            tanh_insts = []
            zs = []
            pe_chain = [None]

            def _pe(inst):
                if pe_chain[0] is not None:
                    add_dep_helper(inst.ins, pe_chain[0].ins, sync=False,
                                   reason="pe order")
                pe_chain[0] = inst
                return inst

            for sl in range(_NSL):
                z = zps.tile([128, _SLICE], f32, tag="z", name=f"z{sl}")
                zs.append(z)
                s_tiles = {}
                pend_z = []

                def _emit_z(u, z=z):
                    s = s_tiles.pop(u)
                    single = (2 * u + 1 >= len(_TILES))
                    if single:
                        _pe(nc.tensor.matmul(
                            z, wcpack_sb[:, u * 256:u * 256 + 128],
                            s[:, 0:_SLICE], start=(u == 0),
                            stop=(u == _N_UNITS - 1)))
                    else:
                        _pe(nc.tensor.matmul(
                            z,
                            wcpack_sb[:, u * 256:(u + 1) * 256].rearrange(
                                "p (j f) -> p j f", j=2),
                            s.rearrange("p (j n) -> p j n", j=2),
                            start=(u == 0), stop=(u == _N_UNITS - 1),
                            perf_mode=DR))

                for q in range(_N_QUADS):
                    qc = _QUAD_CLASSES[q]
                    cb = (sl * _N_QUADS + q) * 2 * _SLICE
                    xwt, lo = _xw_ap(cb, 2 * _SLICE)
                    ntiles = min(4, len(_TILES) - 4 * q)
                    # conv wall: up to 4 concurrent DoubleRow MMs
                    cpt = []
                    for i in range(ntiles):
                        t = _TILES[4 * q + i]
                        kh = _kh((t[1], t[3]))
                        cp = cps.tile([128, _SLICE], f32, tag="cp",
                                      name=f"cp{sl}_{4 * q + i}")
                        cpt.append(cp)
                        _pe(nc.tensor.matmul(
                            cp,
                            w1pack_sb[32 * i:32 * i + kh,
                                      qc * 256:(qc + 1) * 256].rearrange(
                                          "p (j f) -> p j f", j=2),
                            xwt[32 * i:32 * i + kh,
                                lo:lo + 2 * _SLICE].rearrange(
                                    "p (j n) -> p j n", j=2),
                            start=True, stop=True, perf_mode=DR,
                            tile_position=(32 * i, 0)))
                    # nonlinearity per tile, pair-shaped s output
                    for h in range((ntiles + 1) // 2):
                        u = 2 * q + h
                        s = wpool.tile([128, 2 * _SLICE], f8, tag="s",
                                       name=f"s{sl}_{u}", bufs=8)
                        s_tiles[u] = s
                        for jj in range(min(2, ntiles - 2 * h)):
                            ti = 4 * q + 2 * h + jj
                            cp = cpt[2 * h + jj]
                            dst = s[:, jj * _SLICE:(jj + 1) * _SLICE]
                            if _TILE_ON_ACT[ti]:
                                ti_ = nc.scalar.activation(
                                    dst, cp, AF.Tanh,
                                    bias=biasact_sb[:], scale=F["tg"] / _A_IN)
                                tanh_insts.append(ti_)
                            else:
                                nc.vector._custom_dve(
                                    sigpoly, out=dst, in0=cp,
                                    in1=cvec_sb[:, 0:1], s0=biasdve_sb[:],
                                    s1=F["c3"], imm2=F["c1"])
                        pend_z.append(u)
                    # z delayed by two quads so s is always ready
                    while len(pend_z) > 4:
                        _emit_z(pend_z.pop(0))
                while pend_z:
                    _emit_z(pend_z.pop(0))

            # ---- tail: h = (sigma-0.5) via quintic, fc2, log_softmax ----
            last_tanh = tanh_insts[-1]
            for sl in range(_NSL):
                hp = wpool.tile([128, _SLICE], f16, tag="h", name=f"h{sl}")
                nc.vector._custom_dve(
                    sigpoly, out=hp, in0=zs[sl], in1=cvec_sb[:, 1:2],
                    s0=bch_sb[:], s1=F["a3"] / _G ** 3, imm2=F["a1"] / _G)
                ng = _SLICE // 128
                fp = fps.tile([128, 10 * ng], f32, tag="fp", name=f"fp{sl}",
                              bufs=1)
                for g in range(ng):
                    nc.tensor.matmul(fp[:, g * 10:(g + 1) * 10],
                                     hp[:, g * 128:(g + 1) * 128],
                                     fw2t_sb[:], start=True, stop=True)
                lg = wpool.tile([128, 10 * ng], f32, tag="lg", name=f"lg{sl}")
                nc.vector.tensor_tensor(out=lg, in0=fp, in1=fb2r_sb[:, 0:10 * ng],
                                        op=AluOpType.add)
                e = wpool.tile([128, 10 * ng], f32, tag="e", name=f"e{sl}")
                ei = nc.scalar.activation(e, lg, AF.Exp)
                add_dep_helper(ei.ins, last_tanh.ins, sync=False,
                               reason="exp after last tanh (table sets)")
                ssum = wpool.tile([128, ng], f32, tag="ss", name=f"ss{sl}")
                nc.vector.tensor_reduce(
                    ssum, e.rearrange("p (g k) -> p g k", k=10),
                    axis=mybir.AxisListType.X, op=AluOpType.add)
                lns = wpool.tile([128, ng], f32, tag="ls", name=f"ls{sl}")
                li = nc.scalar.activation(lns, ssum, AF.Ln)
                add_dep_helper(li.ins, last_tanh.ins, sync=False,
                               reason="ln after last tanh (table sets)")
                ot = wpool.tile([128, 10 * ng], f32, tag="ot", name=f"ot{sl}")
                for g in range(ng):
                    nc.vector.tensor_scalar(
                        out=ot[:, g * 10:(g + 1) * 10],
                        in0=lg[:, g * 10:(g + 1) * 10],
                        scalar1=lns[:, g:g + 1], scalar2=None,
                        op0=AluOpType.subtract)
                orow = sl * _SLICE
                nc.sync.dma_start(
                    out=out_d[orow:orow + _SLICE, :].rearrange(
                        "(g p) k -> p g k", p=128),
                    in_=ot.rearrange("p (g k) -> p g k", k=10))
    nc.compile()
    return nc


_PROGRAM_CACHE = {}


def kernel(x, w1, b1, w2, b2, fw1, fb1, fw2, fb2):
    global LAST_RESULTS
    slabs, consts = _host_prep(x, w1, b1, w2, b2, fw1, fb1, fw2, fb2)

    if "nc" not in _PROGRAM_CACHE:
        _PROGRAM_CACHE["nc"] = _build_program()
    nc = _PROGRAM_CACHE["nc"]

    in_maps = []
    for c in range(_NCORES):
        m = dict(consts)
        m["xwin"] = slabs[c]
        in_maps.append(m)

    from concourse.bass_utils import run_bass_kernel_spmd
    trace = bool(int(os.environ.get("BASS_KERNEL_TRACE", "0")))
    res = run_bass_kernel_spmd(nc, in_maps, core_ids=list(range(_NCORES)),
                               trace=trace)
    LAST_RESULTS = res
    return np.concatenate([r["out"] for r in res.results], axis=0)
